# revision 1
# baseline (speedup 1.0000x reference)
"""Sparse-attention Trainium2 kernel, 8-way data-parallel over batch.

Reference computation (per batch):
  qkv = x @ qkv_w.T + qkv_b              -> split q,k,v [H=12, N=388, D=64]
  template queries (tokens 0:128) attend to template keys (0:128)
  search queries (tokens 128:388) attend to all 388 keys
  out = concat @ proj_w.T + proj_b

Kernel strategy per core (B_local=8 batches, all compute on device, bf16
matmuls with fp32 PSUM accumulation):
  - x cast to bf16, transposed feature-major via DMA-xbar transposes.
  - q^T,k^T = W^T-stationary matmuls (feature-major out, per-partition bias
    added in fp32 on ACT during the PSUM->SBUF copy).
  - v = x^T-stationary matmuls (token-major out), stored per-head with a ones
    column appended so the attention-value matmul also produces softmax sums.
  - scores computed TRANSPOSED: S^T[k,q] = k^T-slices as lhsT, q^T as rhs.
    exp on ACT (scale=1/8 folded in), probs in bf16.
  - AV: out^T[d,q] accumulated over k-chunks; row 64 = softmax denominators.
  - normalize: reciprocal on DVE, partition_broadcast on GpSimd, multiply on
    DVE (writes bf16 feature-major attention output).
  - proj matmul reads attention output directly (no transposes), bias on DVE,
    DMA out token-major fp32.
"""

import numpy as np

B, N, C = 64, 388, 768
H, D = 12, 64
LT = 128          # template tokens (= first token chunk, exactly)
LS = N - LT       # 260 search tokens
NCORES = 8
BL = B // NCORES  # 8 batches per core
O3 = 3 * C        # 2304
SCALE = 0.125
NPAD = 416        # 388 tokens padded to 32-multiple for DMA-transpose tiles

_NC_CACHE = {}


def _build_nc(dump=False, reps=1, skip=()):
    from contextlib import ExitStack

    import concourse.tile as tile
    from concourse import bacc, mybir
    from concourse.masks import make_identity

    f32 = mybir.dt.float32
    bf16 = mybir.dt.bfloat16
    Identity = mybir.ActivationFunctionType.Identity
    Exp = mybir.ActivationFunctionType.Exp
    mult = mybir.AluOpType.mult
    add = mybir.AluOpType.add

    nc = bacc.Bacc("TRN2", target_bir_lowering=False)

    x_ext = nc.dram_tensor("x", [BL, N, C], f32, kind="ExternalInput")
    qkvw_ext = nc.dram_tensor("qkv_w", [O3, C], f32, kind="ExternalInput")
    qkvb_ext = nc.dram_tensor("qkv_b", [O3], f32, kind="ExternalInput")
    projw_ext = nc.dram_tensor("proj_w", [C, C], f32, kind="ExternalInput")
    projb_ext = nc.dram_tensor("proj_b", [C], f32, kind="ExternalInput")
    out_ext = nc.dram_tensor("out", [BL, N, C], f32, kind="ExternalOutput")
    if dump:
        dxt_ext = nc.dram_tensor("d_xt", [128, 6, N], bf16, kind="ExternalOutput")
        dq_ext = nc.dram_tensor("d_q", [128, 6, N], bf16, kind="ExternalOutput")
        dk_ext = nc.dram_tensor("d_k", [128, 6, N], bf16, kind="ExternalOutput")
        dv_ext = nc.dram_tensor("d_v", [128, 4, H, 65], bf16, kind="ExternalOutput")
        da_ext = nc.dram_tensor("d_at", [128, 6, N], bf16, kind="ExternalOutput")
        dpav_ext = nc.dram_tensor("d_pav", [65, N], f32, kind="ExternalOutput")
        drb_ext = nc.dram_tensor("d_rb", [64, N], f32, kind="ExternalOutput")

    # token chunking of the 388 tokens: 128,128,128,4
    TCH = [(0, 128), (128, 128), (256, 128), (384, 4)]

    with tile.TileContext(nc) as tc, ExitStack() as ctx:
        const = ctx.enter_context(tc.tile_pool(name="const", bufs=1))
        stage = ctx.enter_context(tc.tile_pool(name="stage", bufs=4))
        psum = ctx.enter_context(tc.tile_pool(name="ps", bufs=8, space="PSUM"))

        ident = const.tile([128, 128], f32)
        make_identity(nc, ident)
        ident_b = const.tile([128, 128], bf16)
        make_identity(nc, ident_b)

        # ---- weights/biases declared here; emission interleaved with batch 0
        wT = const.tile([128, 6, O3], bf16)
        projT = const.tile([128, 6, C], bf16)
        qkb_sb = const.tile([128, 12], f32)
        vb_bc = const.tile([128, C], f32)
        pb_bc = const.tile([128, C], f32)

        def weights_gen():
            qb_st = stage.tile([12, 128], f32, tag="bst")
            nc.sync.dma_start(out=qb_st[:], in_=qkvb_ext[0:1536].rearrange("(j p) -> j p", p=128))
            pbt = psum.tile([128, 12], f32, tag="ps")
            nc.tensor.transpose(pbt[:], qb_st[:], ident[0:12, 0:12])
            nc.scalar.copy(out=qkb_sb[:], in_=pbt[:])
            # weight chunks: PE-transpose fp32, cast to bf16 on PSUM->SBUF copy
            for j in range(24):
                if j == 4:
                    nc.sync.dma_start(out=vb_bc[:], in_=qkvb_ext[1536:2304].unsqueeze(0).to_broadcast([128, C]))
                    nc.sync.dma_start(out=pb_bc[:], in_=projb_ext[:].unsqueeze(0).to_broadcast([128, C]))
                wstf = stage.tile([128, C], f32, tag="wstf")
                src = qkvw_ext[j * 128:(j + 1) * 128, :] if j < 18 else \
                    projw_ext[(j - 18) * 128:(j - 17) * 128, :]
                nc.sync.dma_start(out=wstf[:], in_=src)
                wdst = wT if j < 18 else projT
                jo = j * 128 if j < 18 else (j - 18) * 128
                for ct0, ng in ((0, 4), (4, 2)):
                    pt = psum.tile([128, ng * 128], f32, tag="ps")
                    for g in range(ng):
                        ct = ct0 + g
                        nc.tensor.transpose(pt[:, g * 128:(g + 1) * 128],
                                            wstf[:, ct * 128:(ct + 1) * 128], ident[:])
                    dst = wdst[:, ct0:ct0 + ng, jo:jo + 128]
                    src_ap = pt[:].rearrange("p (g t) -> p g t", g=ng)
                    if ct0 == 0:
                        nc.scalar.copy(out=dst, in_=src_ap)
                    else:
                        nc.vector.tensor_copy(out=dst, in_=src_ap)
                yield

        # ---- per-batch pools ----
        xpool = ctx.enter_context(tc.tile_pool(name="xp", bufs=2))
        xtpool = ctx.enter_context(tc.tile_pool(name="xtp", bufs=2))
        qkpool = ctx.enter_context(tc.tile_pool(name="qkp", bufs=2))
        vpool = ctx.enter_context(tc.tile_pool(name="vp", bufs=2))
        apool = ctx.enter_context(tc.tile_pool(name="ap", bufs=2))
        ppool = ctx.enter_context(tc.tile_pool(name="pp", bufs=4))
        spool = ctx.enter_context(tc.tile_pool(name="ssp", bufs=4))
        opool = ctx.enter_context(tc.tile_pool(name="op", bufs=3))

        def emit_xload(b):
            xf = xpool.tile([128, 4, C], f32, tag="xf")
            nc.sync.dma_start(
                out=xf[:, 0:3, :],
                in_=x_ext[b, 0:384, :].rearrange("(t p) c -> p t c", p=128),
            )
            nc.sync.dma_start(out=xf[0:4, 3, :], in_=x_ext[b, 384:388, :])
            return xf

        def stage1(b, xf, st):
            """Generator: transposes (4 items), q/k groups (12), v halves (8).
            Yields between PE-work units so attention of the previous batch
            can interleave. Fills `st` with the batch's tiles."""
            xTb = xtpool.tile([128, 6, N], bf16, tag="xt")
            st["xT"] = xTb
            xc = xpool.tile([128, 4, C], bf16, tag="xc")
            for ti, (t0, tp) in enumerate(TCH):
                # cast this chunk to bf16 on ACT, then 1-cycle/row transposes
                nc.scalar.copy(out=xc[0:tp, ti, :], in_=xf[0:tp, ti, :])
                for ct0, ng in ((0, 4), (4, 2)):
                    pt = psum.tile([128, ng * tp], bf16, tag="ps")
                    for g in range(ng):
                        ct = ct0 + g
                        nc.tensor.transpose(pt[:, g * tp:(g + 1) * tp],
                                            xc[0:tp, ti, ct * 128:(ct + 1) * 128],
                                            ident_b[0:tp, 0:tp])
                    dst = xTb[:, ct0:ct0 + ng, t0:t0 + tp]
                    src_ap = pt[:].rearrange("p (g t) -> p g t", g=ng)
                    if ct0 == 0:
                        nc.vector.tensor_copy(out=dst, in_=src_ap)
                    else:
                        nc.scalar.copy(out=dst, in_=src_ap)
                yield

            qTb = qkpool.tile([128, 6, N], bf16, tag="q")
            kTb = qkpool.tile([128, 6, N], bf16, tag="k")
            st["q"], st["k"] = qTb, kTb
            if "qkv" in skip:
                nc.vector.memset(qTb[:, 0:1, 0:2], 0.0)
                nc.vector.memset(kTb[:, 0:1, 0:2], 0.0)
            for j in range(12 if "qkv" not in skip else 0):
                ps = psum.tile([128, N], f32, tag="ps")
                for ct in range(6):
                    nc.tensor.matmul(
                        ps[:],
                        lhsT=wT[:, ct, j * 128:(j + 1) * 128],
                        rhs=xTb[:, ct, :],
                        start=(ct == 0), stop=(ct == 5),
                    )
                dst = qTb[:, j, :] if j < 6 else kTb[:, j - 6, :]
                if j % 2 == 0:
                    nc.scalar.activation(out=dst, in_=ps[:], func=Identity,
                                         bias=qkb_sb[:, j:j + 1], scale=1.0)
                else:
                    nc.vector.tensor_scalar(out=dst, in0=ps[:],
                                            scalar1=qkb_sb[:, j:j + 1], scalar2=None,
                                            op0=add)
                yield

            # block-diagonal remainder tiles for the 4 leftover key tokens:
            # kTrem[:, cth, 0:4] = even head's k-remainder (d-rows 0:64),
            # kTrem[:, cth, 32:36] = odd head's (d-rows 64:128); other columns
            # zero so one matmul yields both heads' remainder scores.
            kTrem = qkpool.tile([128, 6, 36], bf16, tag="krem")
            st["krem"] = kTrem
            nc.vector.memset(kTrem[:], 0.0)
            nc.vector.tensor_copy(out=kTrem[0:64, :, 0:4], in_=kTb[0:64, :, 384:388])
            nc.vector.tensor_copy(out=kTrem[64:128, :, 32:36], in_=kTb[64:128, :, 384:388])

            vb = vpool.tile([128, 4, H, 65], bf16, tag="v")
            st["v"] = vb
            nc.vector.memset(vb[:, :, :, 64:65], 1.0)
            for ti, (t0, tp) in enumerate(TCH if "qkv" not in skip else []):
                for o0, on, hs, he in ((0, 512, 0, 8), (512, 256, 8, 12)):
                    pv = psum.tile([128, on], f32, tag="ps")
                    for ct in range(6):
                        nc.tensor.matmul(
                            pv[0:tp, 0:on],
                            lhsT=xTb[:, ct, t0:t0 + tp],
                            rhs=wT[:, ct, 1536 + o0:1536 + o0 + on],
                            start=(ct == 0), stop=(ct == 5),
                        )
                    nc.vector.tensor_tensor(
                        out=vb[0:tp, ti, hs:he, 0:64],
                        in0=pv[0:tp, :].rearrange("p (h d) -> p h d", h=he - hs),
                        in1=vb_bc[0:tp, o0:o0 + on].rearrange("p (h d) -> p h d", h=he - hs),
                        op=add,
                    )
                    yield

            # v-remainder regrouped to match kTrem's partition layout
            vrem = vpool.tile([36, 6, 65], bf16, tag="vrem")
            st["vrem"] = vrem
            if "qkv" not in skip:
                nc.vector.tensor_copy(out=vrem[0:4, :, :], in_=vb[0:4, 3, 0:12:2, :])
                nc.vector.tensor_copy(out=vrem[32:36, :, :], in_=vb[0:4, 3, 1:12:2, :])
            else:
                nc.vector.memset(vrem[:], 0.0)
                nc.vector.memset(vb[:, :, :, 0:64], 0.0)

        def emit_attention(b, st, filler):
            """Attention heads; pulls filler items between scores and AVs."""
            qTb, kTb, vb = st["q"], st["k"], st["v"]
            kTrem, vrem = st["krem"], st["vrem"]
            xattnT = apool.tile([128, 6, N], bf16, tag="xat")
            st["at"] = xattnT
            if "attn" in skip:
                for _ct in range(6):
                    nc.vector.tensor_copy(out=xattnT[:, _ct, :], in_=wT[:, 0, 0:N])
            nheads = H if "attn" not in skip else 0
            pulled = 0
            for h in range(nheads):
                cth, r0 = h // 2, (h % 2) * 64
                qh = qTb[r0:r0 + 64, cth, :]   # [64, 388] bf16
                kh = kTb[r0:r0 + 64, cth, :]

                # all scores matmuls first; chunk 0 covers ALL queries
                # (template cols 0:128 + search 128:388); chunk 3 (the 4
                # remainder keys) is computed for the HEAD PAIR at even h via
                # the block-diagonal kTrem in one matmul + one exp
                probs = []
                for kc, (t0, tp) in list(enumerate(TCH))[0:3]:
                    pss = psum.tile([128, N if kc == 0 else LS], f32, tag="ps")
                    rhs_q = qh[:] if kc == 0 else qh[:, LT:N]
                    nc.tensor.matmul(pss[0:tp, :], lhsT=kh[:, t0:t0 + tp],
                                     rhs=rhs_q, start=True, stop=True)
                    prs = ppool.tile([128, N if kc == 0 else LS], bf16,
                                     tag="pr0" if kc == 0 else "prs")
                    nc.scalar.activation(out=prs[0:tp, :], in_=pss[0:tp, :],
                                         func=(Identity if "expid" in skip else Exp),
                                         scale=SCALE)
                    probs.append(prs)
                if h % 2 == 0:
                    psr = psum.tile([36, LS], f32, tag="ps")
                    nc.tensor.matmul(psr[:], lhsT=kTrem[:, cth, :],
                                     rhs=qTb[:, cth, LT:N], start=True, stop=True)
                    prr = ppool.tile([36, LS], bf16, tag="prr")
                    nc.scalar.activation(out=prr[:], in_=psr[:],
                                         func=(Identity if "expid" in skip else Exp),
                                         scale=SCALE)
                    st["prr"] = prr
                else:
                    prr = st["prr"]

                # filler work for neighbouring batches rides in the exp window
                want = (h + 1) * 22 // nheads
                while pulled < want and next(filler, "END") != "END":
                    pulled += 1

                pav = psum.tile([65, N], f32, tag="ps")
                nc.tensor.matmul(pav[:, 0:N], lhsT=vb[:, 0, h, :],
                                 rhs=probs[0][:, 0:N], start=True, stop=False)
                for kc, (t0, tp) in list(enumerate(TCH))[1:3]:
                    nc.tensor.matmul(pav[:, LT:N], lhsT=vb[0:tp, kc, h, :],
                                     rhs=probs[kc][0:tp, :],
                                     start=False, stop=False)
                rr = (h % 2) * 32
                nc.tensor.matmul(pav[:, LT:N], lhsT=vrem[rr:rr + 4, cth, :],
                                 rhs=prr[rr:rr + 4, :], start=False, stop=True)

                if dump and b == 0 and h == 0:
                    pavf = spool.tile([65, N], f32, tag="pavf")
                    nc.vector.tensor_copy(out=pavf[:], in_=pav[:])
                    nc.sync.dma_start(out=dpav_ext[:], in_=pavf[:])
                if "epi" in skip:
                    nc.vector.tensor_copy(out=xattnT[r0:r0 + 64, cth, :],
                                          in_=pav[0:64, :])
                else:
                    rinv = spool.tile([1, N], f32, tag="ri")
                    nc.vector.reciprocal(out=rinv[:], in_=pav[64:65, :])
                    rb = spool.tile([64, N], f32, tag="rb")
                    nc.gpsimd.partition_broadcast(rb[:], rinv[:])
                    if dump and b == 0 and h == 0:
                        nc.sync.dma_start(out=drb_ext[:], in_=rb[:])
                    nc.vector.tensor_tensor(out=xattnT[r0:r0 + 64, cth, :],
                                            in0=pav[0:64, :], in1=rb[:], op=mult)

            if dump and b == 0:
                nc.sync.dma_start(out=dxt_ext[:], in_=st["xT"][:, :, 0:N])
                nc.sync.dma_start(out=dq_ext[:], in_=qTb[:])
                nc.sync.dma_start(out=dk_ext[:], in_=kTb[:])
                nc.sync.dma_start(out=dv_ext[:], in_=vb[:])
                nc.sync.dma_start(out=da_ext[:], in_=xattnT[:])
            # drain any remaining filler
            while next(filler, "END") != "END":
                pass

        def proj_gen(b, st):
            """Generator: 4 proj+store chunk items."""
            xattnT = st["at"]
            for ti, (t0, tp) in enumerate(TCH if "proj" not in skip else []):
                osb = opool.tile([128, C], f32, tag="ob")
                for o0, on in ((0, 512), (512, 256)):
                    pp = psum.tile([128, on], f32, tag="ps")
                    for ct in range(6):
                        nc.tensor.matmul(
                            pp[0:tp, 0:on],
                            lhsT=xattnT[:, ct, t0:t0 + tp],
                            rhs=projT[:, ct, o0:o0 + on],
                            start=(ct == 0), stop=(ct == 5),
                        )
                    nc.vector.tensor_tensor(out=osb[0:tp, o0:o0 + on], in0=pp[0:tp, :],
                                            in1=pb_bc[0:tp, o0:o0 + on], op=add)
                nc.sync.dma_start(out=out_ext[b, t0:t0 + tp, :], in_=osb[0:tp, :])
                yield

        # ---- software-pipelined batch loop ----
        from itertools import chain

        seq = [bb for _ in range(reps) for bb in range(BL)]
        states = [dict() for _ in seq]
        # prologue: interleave the 24 weight-prep chunks with batch 0's
        # stage1 items (transposes need no weights; qk group j needs weight
        # chunk j which is already emitted by then; v needs chunks 12-17)
        gw = weights_gen()
        xf0 = emit_xload(seq[0])
        g0 = stage1(seq[0], xf0, states[0])
        for _ in range(24):
            next(gw, None)
            next(g0, None)
        for _ in g0:
            pass
        prev_proj = iter(())
        for i, b in enumerate(seq):
            if i + 1 < len(seq):
                xf_n = emit_xload(seq[i + 1])
                nxt = stage1(seq[i + 1], xf_n, states[i + 1])
            else:
                nxt = iter(())
            emit_attention(b, states[i], chain(prev_proj, nxt))
            prev_proj = proj_gen(b, states[i])
        for _ in prev_proj:
            pass

    nc.compile()
    return nc


def _get_nc():
    if "nc" not in _NC_CACHE:
        _NC_CACHE["nc"] = _build_nc()
    return _NC_CACHE["nc"]


def kernel(x, qkv_w, qkv_b, proj_w, proj_b, t_h=8, t_w=8, s_h=16, s_w=16):
    from concourse.bass_utils import run_bass_kernel_spmd

    x = np.ascontiguousarray(np.asarray(x, dtype=np.float32))
    qkv_w = np.ascontiguousarray(np.asarray(qkv_w, dtype=np.float32))
    qkv_b = np.ascontiguousarray(np.asarray(qkv_b, dtype=np.float32))
    proj_w = np.ascontiguousarray(np.asarray(proj_w, dtype=np.float32))
    proj_b = np.ascontiguousarray(np.asarray(proj_b, dtype=np.float32))

    nc = _get_nc()
    in_maps = [
        {
            "x": x[i * BL:(i + 1) * BL],
            "qkv_w": qkv_w,
            "qkv_b": qkv_b,
            "proj_w": proj_w,
            "proj_b": proj_b,
        }
        for i in range(NCORES)
    ]
    res = run_bass_kernel_spmd(nc, in_maps, core_ids=list(range(NCORES)))
    out = np.concatenate([res.results[i]["out"] for i in range(NCORES)], axis=0)
    return out.astype(np.float32)



# revision 14
# speedup vs baseline: 1.6825x; 1.6825x over previous
"""Sparse-attention Trainium2 kernel, 8-way data-parallel over batch.

Reference computation (per batch):
  qkv = x @ qkv_w.T + qkv_b              -> split q,k,v [H=12, N=388, D=64]
  template queries (tokens 0:128) attend to template keys (0:128)
  search queries (tokens 128:388) attend to all 388 keys
  out = concat @ proj_w.T + proj_b

Kernel strategy per core (B_local=8 batches, all compute on device, bf16
matmuls with fp32 PSUM accumulation):
  - x cast to bf16, transposed feature-major via DMA-xbar transposes.
  - q^T,k^T = W^T-stationary matmuls (feature-major out, per-partition bias
    added in fp32 on ACT during the PSUM->SBUF copy).
  - v = x^T-stationary matmuls (token-major out), stored per-head with a ones
    column appended so the attention-value matmul also produces softmax sums.
  - scores computed TRANSPOSED: S^T[k,q] = k^T-slices as lhsT, q^T as rhs.
    exp on ACT (scale=1/8 folded in), probs in bf16.
  - AV: out^T[d,q] accumulated over k-chunks; row 64 = softmax denominators.
  - normalize: reciprocal on DVE, partition_broadcast on GpSimd, multiply on
    DVE (writes bf16 feature-major attention output).
  - proj matmul reads attention output directly (no transposes), bias on DVE,
    DMA out token-major fp32.
"""

import numpy as np

B, N, C = 64, 388, 768
H, D = 12, 64
LT = 128          # template tokens (= first token chunk, exactly)
LS = N - LT       # 260 search tokens
NCORES = 8
BL = B // NCORES  # 8 batches per core
O3 = 3 * C        # 2304
SCALE = 0.125
NPAD = 416        # 388 tokens padded to 32-multiple for DMA-transpose tiles

_NC_CACHE = {}


def _build_nc(dump=False, reps=1, skip=()):
    from contextlib import ExitStack

    import concourse.tile as tile
    from concourse import bacc, mybir
    from concourse.masks import make_identity

    f32 = mybir.dt.float32
    bf16 = mybir.dt.bfloat16
    Identity = mybir.ActivationFunctionType.Identity
    Exp = mybir.ActivationFunctionType.Exp
    mult = mybir.AluOpType.mult
    add = mybir.AluOpType.add

    nc = bacc.Bacc("TRN2", target_bir_lowering=False)

    x_ext = nc.dram_tensor("x", [BL, N, C], f32, kind="ExternalInput")
    qkvw_ext = nc.dram_tensor("qkv_w", [O3, C], f32, kind="ExternalInput")
    qkvb_ext = nc.dram_tensor("qkv_b", [O3], f32, kind="ExternalInput")
    projw_ext = nc.dram_tensor("proj_w", [C, C], f32, kind="ExternalInput")
    projb_ext = nc.dram_tensor("proj_b", [C], f32, kind="ExternalInput")
    out_ext = nc.dram_tensor("out", [BL, N, C], f32, kind="ExternalOutput")
    if dump:
        dxt_ext = nc.dram_tensor("d_xt", [128, 6, N], bf16, kind="ExternalOutput")
        dq_ext = nc.dram_tensor("d_q", [128, 6, N], bf16, kind="ExternalOutput")
        dk_ext = nc.dram_tensor("d_k", [128, 6, N], bf16, kind="ExternalOutput")
        dv_ext = nc.dram_tensor("d_v", [128, 4, H, 65], bf16, kind="ExternalOutput")
        da_ext = nc.dram_tensor("d_at", [128, 6, N], bf16, kind="ExternalOutput")
        dpav_ext = nc.dram_tensor("d_pav", [65, N], f32, kind="ExternalOutput")
        drb_ext = nc.dram_tensor("d_rb", [64, N], f32, kind="ExternalOutput")

    # token chunking of the 388 tokens: 128,128,128,4
    TCH = [(0, 128), (128, 128), (256, 128), (384, 4)]

    with tile.TileContext(nc) as tc, ExitStack() as ctx:
        const = ctx.enter_context(tc.tile_pool(name="const", bufs=1))
        stage = ctx.enter_context(tc.tile_pool(name="stage", bufs=4))
        # 8 PSUM banks total: 5 general + 2 deferred-AV accumulators + 1
        # reciprocal-broadcast target
        psum = ctx.enter_context(tc.tile_pool(name="ps", bufs=5, space="PSUM"))
        pavpool = ctx.enter_context(tc.tile_pool(name="pav", bufs=2, space="PSUM"))
        pbcpool = ctx.enter_context(tc.tile_pool(name="pbc", bufs=1, space="PSUM"))

        ident = const.tile([128, 128], f32)
        make_identity(nc, ident)
        ident_b = const.tile([128, 128], bf16)
        make_identity(nc, ident_b)
        ones1 = const.tile([1, 64], bf16)
        nc.vector.memset(ones1[:], 1.0)

        # ---- weights/biases declared here; emission interleaved with batch 0
        wT = const.tile([128, 6, O3], bf16)
        projT = const.tile([128, 6, C], bf16)
        qkb_sb = const.tile([128, 12], f32)
        vb_bc = const.tile([128, C], f32)
        pb_bc = const.tile([128, C], f32)

        def weights_gen():
            qb_st = stage.tile([12, 128], f32, tag="bst")
            nc.sync.dma_start(out=qb_st[:], in_=qkvb_ext[0:1536].rearrange("(j p) -> j p", p=128))
            pbt = psum.tile([128, 12], f32, tag="ps")
            nc.tensor.transpose(pbt[:], qb_st[:], ident[0:12, 0:12])
            nc.scalar.copy(out=qkb_sb[:], in_=pbt[:])
            # weight chunks: PE-transpose fp32, cast to bf16 on PSUM->SBUF copy
            for j in range(24):
                if j == 4:
                    nc.sync.dma_start(out=vb_bc[:], in_=qkvb_ext[1536:2304].unsqueeze(0).to_broadcast([128, C]))
                    nc.sync.dma_start(out=pb_bc[:], in_=projb_ext[:].unsqueeze(0).to_broadcast([128, C]))
                wstf = stage.tile([128, C], f32, tag="wstf")
                src = qkvw_ext[j * 128:(j + 1) * 128, :] if j < 18 else \
                    projw_ext[(j - 18) * 128:(j - 17) * 128, :]
                nc.sync.dma_start(out=wstf[:], in_=src)
                wdst = wT if j < 18 else projT
                jo = j * 128 if j < 18 else (j - 18) * 128
                for ct0, ng in ((0, 4), (4, 2)):
                    pt = psum.tile([128, ng * 128], f32, tag="ps")
                    for g in range(ng):
                        ct = ct0 + g
                        nc.tensor.transpose(pt[:, g * 128:(g + 1) * 128],
                                            wstf[:, ct * 128:(ct + 1) * 128], ident[:])
                    dst = wdst[:, ct0:ct0 + ng, jo:jo + 128]
                    src_ap = pt[:].rearrange("p (g t) -> p g t", g=ng)
                    if ct0 == 0:
                        nc.scalar.copy(out=dst, in_=src_ap)
                    else:
                        nc.vector.tensor_copy(out=dst, in_=src_ap)
                yield

        # ---- per-batch pools ----
        xpool = ctx.enter_context(tc.tile_pool(name="xp", bufs=2))
        xtpool = ctx.enter_context(tc.tile_pool(name="xtp", bufs=2))
        qkpool = ctx.enter_context(tc.tile_pool(name="qkp", bufs=2))
        vpool = ctx.enter_context(tc.tile_pool(name="vp", bufs=2))
        apool = ctx.enter_context(tc.tile_pool(name="ap", bufs=2))
        ppool = ctx.enter_context(tc.tile_pool(name="pp", bufs=4))
        spool = ctx.enter_context(tc.tile_pool(name="ssp", bufs=4))
        opool = ctx.enter_context(tc.tile_pool(name="op", bufs=3))

        def emit_xload(b):
            # one DMA per 128-token chunk: the first transpose only waits for
            # chunk 0 (~1/3 of the full-x DMA time)
            xf = xpool.tile([128, 4, C], f32, tag="xf")
            for ti in range(3):
                nc.sync.dma_start(out=xf[:, ti, :],
                                  in_=x_ext[b, ti * 128:(ti + 1) * 128, :])
            nc.sync.dma_start(out=xf[0:4, 3, :], in_=x_ext[b, 384:388, :])
            return xf

        def stage1(b, xf, st):
            """Generator: transposes (4 items), q/k groups (12), v halves (8).
            Yields between PE-work units so attention of the previous batch
            can interleave. Fills `st` with the batch's tiles."""
            xTb = xtpool.tile([128, 6, N], bf16, tag="xt")
            st["xT"] = xTb
            xc = xpool.tile([128, 4, C], bf16, tag="xc")
            for ti, (t0, tp) in enumerate(TCH):
                # cast this chunk to bf16 on ACT, then 1-cycle/row transposes
                nc.scalar.copy(out=xc[0:tp, ti, :], in_=xf[0:tp, ti, :])
                for ct0, ng in ((0, 4), (4, 2)):
                    pt = psum.tile([128, ng * tp], bf16, tag="ps")
                    for g in range(ng):
                        ct = ct0 + g
                        nc.tensor.transpose(pt[:, g * tp:(g + 1) * tp],
                                            xc[0:tp, ti, ct * 128:(ct + 1) * 128],
                                            ident_b[0:tp, 0:tp])
                    dst = xTb[:, ct0:ct0 + ng, t0:t0 + tp]
                    src_ap = pt[:].rearrange("p (g t) -> p g t", g=ng)
                    if ct0 == 0:
                        nc.vector.tensor_copy(out=dst, in_=src_ap)
                    else:
                        nc.scalar.copy(out=dst, in_=src_ap)
                yield

            qTb = qkpool.tile([128, 6, N], bf16, tag="q")
            kTb = qkpool.tile([128, 6, N], bf16, tag="k")
            st["q"], st["k"] = qTb, kTb
            if "qkv" in skip:
                nc.vector.memset(qTb[:, 0:1, 0:2], 0.0)
                nc.vector.memset(kTb[:, 0:1, 0:2], 0.0)
            for j in range(12 if "qkv" not in skip else 0):
                ps = psum.tile([128, N], f32, tag="ps")
                for ct in range(6):
                    nc.tensor.matmul(
                        ps[:],
                        lhsT=wT[:, ct, j * 128:(j + 1) * 128],
                        rhs=xTb[:, ct, :],
                        start=(ct == 0), stop=(ct == 5),
                    )
                dst = qTb[:, j, :] if j < 6 else kTb[:, j - 6, :]
                if j % 2 == 0:
                    nc.scalar.activation(out=dst, in_=ps[:], func=Identity,
                                         bias=qkb_sb[:, j:j + 1], scale=1.0)
                else:
                    nc.vector.tensor_scalar(out=dst, in0=ps[:],
                                            scalar1=qkb_sb[:, j:j + 1], scalar2=None,
                                            op0=add)
                yield

            # block-diagonal remainder tiles for the 4 leftover key tokens:
            # kTrem[:, cth, 0:4] = even head's k-remainder (d-rows 0:64),
            # kTrem[:, cth, 32:36] = odd head's (d-rows 64:128); other columns
            # zero so one matmul yields both heads' remainder scores.
            kTrem = qkpool.tile([128, 6, 36], bf16, tag="krem")
            st["krem"] = kTrem
            nc.vector.memset(kTrem[:], 0.0)
            nc.vector.tensor_copy(out=kTrem[0:64, :, 0:4], in_=kTb[0:64, :, 384:388])
            nc.vector.tensor_copy(out=kTrem[64:128, :, 32:36], in_=kTb[64:128, :, 384:388])

            vb = vpool.tile([128, 4, H, 65], bf16, tag="v")
            st["v"] = vb
            nc.vector.memset(vb[:, :, :, 64:65], 1.0)
            for ti, (t0, tp) in enumerate(TCH if "qkv" not in skip else []):
                for o0, on, hs, he in ((0, 512, 0, 8), (512, 256, 8, 12)):
                    pv = psum.tile([128, on], f32, tag="ps")
                    for ct in range(6):
                        nc.tensor.matmul(
                            pv[0:tp, 0:on],
                            lhsT=xTb[:, ct, t0:t0 + tp],
                            rhs=wT[:, ct, 1536 + o0:1536 + o0 + on],
                            start=(ct == 0), stop=(ct == 5),
                        )
                    nc.vector.tensor_tensor(
                        out=vb[0:tp, ti, hs:he, 0:64],
                        in0=pv[0:tp, :].rearrange("p (h d) -> p h d", h=he - hs),
                        in1=vb_bc[0:tp, o0:o0 + on].rearrange("p (h d) -> p h d", h=he - hs),
                        op=add,
                    )
                    yield

            # v-remainder regrouped to match kTrem's partition layout
            vrem = vpool.tile([36, 6, 65], bf16, tag="vrem")
            st["vrem"] = vrem
            if "qkv" not in skip:
                nc.vector.tensor_copy(out=vrem[0:4, :, :], in_=vb[0:4, 3, 0:12:2, :])
                nc.vector.tensor_copy(out=vrem[32:36, :, :], in_=vb[0:4, 3, 1:12:2, :])
            else:
                nc.vector.memset(vrem[:], 0.0)
                nc.vector.memset(vb[:, :, :, 0:64], 0.0)

        def emit_attention(b, st, filler):
            """Attention heads; pulls filler items between scores and AVs.

            Softmax normalization is deferred one head: reciprocal of the
            denominator row (DVE, bf16) runs under the NEXT head's scores
            matmuls, then a [1,64]-ones PE matmul broadcasts it across 64
            partitions into PSUM, and the DVE multiply normalizes. This
            keeps gpsimd (slow per-op on HW) entirely out of the kernel.
            """
            qTb, kTb, vb = st["q"], st["k"], st["v"]
            kTrem, vrem = st["krem"], st["vrem"]
            xattnT = apool.tile([128, 6, N], bf16, tag="xat")
            st["at"] = xattnT
            if "attn" in skip:
                for _ct in range(6):
                    nc.vector.tensor_copy(out=xattnT[:, _ct, :], in_=wT[:, 0, 0:N])
            nheads = H if "attn" not in skip else 0
            pulled = 0

            def flush_epi(pend):
                pav_p, rinv_p, cth_p, r0_p, dump_p = pend
                pbc = pbcpool.tile([64, N], f32, tag="pbc")
                nc.tensor.matmul(pbc[:], lhsT=ones1[:], rhs=rinv_p[:],
                                 start=True, stop=True)
                # DVE can't read two PSUM operands; stage the broadcast in
                # SBUF via ACT (also keeps DVE free for the multiply)
                rbf = spool.tile([64, N], f32, tag="rbf")
                nc.scalar.copy(out=rbf[:], in_=pbc[:])
                if dump_p:
                    nc.sync.dma_start(out=drb_ext[:], in_=rbf[:])
                nc.vector.tensor_tensor(out=xattnT[r0_p:r0_p + 64, cth_p, :],
                                        in0=pav_p[0:64, :], in1=rbf[:], op=mult)

            def emit_av(h, probs, prr):
                """AV matmuls for head h (scores/exps were emitted one head
                earlier, so the exp outputs are ready — no PE stall)."""
                cth, r0 = h // 2, (h % 2) * 64
                pav = pavpool.tile([65, N], f32, tag="pav")
                nc.tensor.matmul(pav[:, 0:N], lhsT=vb[:, 0, h, :],
                                 rhs=probs[0][:, 0:N], start=True, stop=False)
                for kc, (t0, tp) in list(enumerate(TCH))[1:3]:
                    nc.tensor.matmul(pav[:, LT:N], lhsT=vb[0:tp, kc, h, :],
                                     rhs=probs[kc][0:tp, :],
                                     start=False, stop=False)
                rr = (h % 2) * 32
                nc.tensor.matmul(pav[:, LT:N], lhsT=vrem[rr:rr + 4, cth, :],
                                 rhs=prr[rr:rr + 4, :], start=False, stop=True)

                if dump and b == 0 and h == 0:
                    pavf = spool.tile([65, N], f32, tag="pavf")
                    nc.vector.tensor_copy(out=pavf[:], in_=pav[:])
                    nc.sync.dma_start(out=dpav_ext[:], in_=pavf[:])
                if "epi" in skip:
                    nc.vector.tensor_copy(out=xattnT[r0:r0 + 64, cth, :],
                                          in_=pav[0:64, :])
                    return None
                rinv = spool.tile([1, N], bf16, tag="ri")
                with nc.allow_low_precision(reason="bf16 1/denom feeds a "
                                            "ones-broadcast matmul; ~0.2% "
                                            "rel err, budget is 2e-2"):
                    nc.vector.reciprocal(out=rinv[:], in_=pav[64:65, :])
                return (pav, rinv, cth, r0, dump and b == 0 and h == 0)

            pend = None   # epilogue pending for head h-2
            prev = None   # (h, probs, prr) awaiting AV for head h-1
            for h in range(nheads):
                cth, r0 = h // 2, (h % 2) * 64
                qh = qTb[r0:r0 + 64, cth, :]   # [64, 388] bf16
                kh = kTb[r0:r0 + 64, cth, :]

                # all scores matmuls first; chunk 0 covers ALL queries
                # (template cols 0:128 + search 128:388); chunk 3 (the 4
                # remainder keys) is computed for the HEAD PAIR at even h via
                # the block-diagonal kTrem in one matmul + one exp
                probs = []
                for kc, (t0, tp) in list(enumerate(TCH))[0:3]:
                    pss = psum.tile([128, N if kc == 0 else LS], f32, tag="ps")
                    rhs_q = qh[:] if kc == 0 else qh[:, LT:N]
                    nc.tensor.matmul(pss[0:tp, :], lhsT=kh[:, t0:t0 + tp],
                                     rhs=rhs_q, start=True, stop=True)
                    prs = ppool.tile([128, N if kc == 0 else LS], bf16,
                                     tag="pr0" if kc == 0 else "prs")
                    nc.scalar.activation(out=prs[0:tp, :], in_=pss[0:tp, :],
                                         func=(Identity if "expid" in skip else Exp),
                                         scale=SCALE)
                    probs.append(prs)
                if h % 2 == 0:
                    psr = psum.tile([36, LS], f32, tag="ps")
                    nc.tensor.matmul(psr[:], lhsT=kTrem[:, cth, :],
                                     rhs=qTb[:, cth, LT:N], start=True, stop=True)
                    prr = ppool.tile([36, LS], bf16, tag="prr")
                    nc.scalar.activation(out=prr[:], in_=psr[:],
                                         func=(Identity if "expid" in skip else Exp),
                                         scale=SCALE)
                    st["prr"] = prr
                else:
                    prr = st["prr"]

                # filler work for neighbouring batches rides in the exp window
                want = (h + 1) * 22 // nheads
                while pulled < want and next(filler, "END") != "END":
                    pulled += 1

                # normalize head h-2, then AV for head h-1: both consume
                # results whose producers ran during this head's scores
                if pend is not None:
                    flush_epi(pend)
                    pend = None
                if prev is not None:
                    pend = emit_av(*prev)
                prev = (h, probs, prr)

            # tail: AV of the last head, then the two pending normalizes
            if prev is not None:
                pend2 = emit_av(*prev)
            if pend is not None:
                flush_epi(pend)
            if dump and b == 0:
                nc.sync.dma_start(out=dxt_ext[:], in_=st["xT"][:, :, 0:N])
                nc.sync.dma_start(out=dq_ext[:], in_=qTb[:])
                nc.sync.dma_start(out=dk_ext[:], in_=kTb[:])
                nc.sync.dma_start(out=dv_ext[:], in_=vb[:])
            # drain any remaining filler, then normalize the last head
            while next(filler, "END") != "END":
                pass
            if "attn" not in skip and pend2 is not None:
                flush_epi(pend2)
            if dump and b == 0:
                nc.sync.dma_start(out=da_ext[:], in_=xattnT[:])

        def proj_gen(b, st):
            """Generator: 4 proj+store chunk items."""
            xattnT = st["at"]
            for ti, (t0, tp) in enumerate(TCH if "proj" not in skip else []):
                osb = opool.tile([128, C], f32, tag="ob")
                for o0, on in ((0, 512), (512, 256)):
                    pp = psum.tile([128, on], f32, tag="ps")
                    for ct in range(6):
                        nc.tensor.matmul(
                            pp[0:tp, 0:on],
                            lhsT=xattnT[:, ct, t0:t0 + tp],
                            rhs=projT[:, ct, o0:o0 + on],
                            start=(ct == 0), stop=(ct == 5),
                        )
                    nc.vector.tensor_tensor(out=osb[0:tp, o0:o0 + on], in0=pp[0:tp, :],
                                            in1=pb_bc[0:tp, o0:o0 + on], op=add)
                nc.sync.dma_start(out=out_ext[b, t0:t0 + tp, :], in_=osb[0:tp, :])
                yield

        # ---- software-pipelined batch loop ----
        from itertools import chain

        seq = [bb for _ in range(reps) for bb in range(BL)]
        states = [dict() for _ in seq]
        # prologue: interleave the 24 weight-prep chunks with batch 0's
        # stage1 items (transposes need no weights; qk group j needs weight
        # chunk j which is already emitted by then; v needs chunks 12-17)
        gw = weights_gen()
        xf0 = emit_xload(seq[0])
        g0 = stage1(seq[0], xf0, states[0])
        for _ in range(24):
            next(gw, None)
            next(g0, None)
        for _ in g0:
            pass
        prev_proj = iter(())
        for i, b in enumerate(seq):
            if i + 1 < len(seq):
                xf_n = emit_xload(seq[i + 1])
                nxt = stage1(seq[i + 1], xf_n, states[i + 1])
            else:
                nxt = iter(())
            emit_attention(b, states[i], chain(prev_proj, nxt))
            prev_proj = proj_gen(b, states[i])
        for _ in prev_proj:
            pass

    nc.compile()
    return nc


def _get_nc():
    if "nc" not in _NC_CACHE:
        _NC_CACHE["nc"] = _build_nc()
    return _NC_CACHE["nc"]


def kernel(x, qkv_w, qkv_b, proj_w, proj_b, t_h=8, t_w=8, s_h=16, s_w=16):
    from concourse.bass_utils import run_bass_kernel_spmd

    x = np.ascontiguousarray(np.asarray(x, dtype=np.float32))
    qkv_w = np.ascontiguousarray(np.asarray(qkv_w, dtype=np.float32))
    qkv_b = np.ascontiguousarray(np.asarray(qkv_b, dtype=np.float32))
    proj_w = np.ascontiguousarray(np.asarray(proj_w, dtype=np.float32))
    proj_b = np.ascontiguousarray(np.asarray(proj_b, dtype=np.float32))

    nc = _get_nc()
    in_maps = [
        {
            "x": x[i * BL:(i + 1) * BL],
            "qkv_w": qkv_w,
            "qkv_b": qkv_b,
            "proj_w": proj_w,
            "proj_b": proj_b,
        }
        for i in range(NCORES)
    ]
    res = run_bass_kernel_spmd(nc, in_maps, core_ids=list(range(NCORES)))
    out = np.concatenate([res.results[i]["out"] for i in range(NCORES)], axis=0)
    return out.astype(np.float32)



# revision 23
# speedup vs baseline: 1.8249x; 1.0847x over previous
"""Sparse-attention Trainium2 kernel, 8-way data-parallel over batch.

Reference computation (per batch):
  qkv = x @ qkv_w.T + qkv_b              -> split q,k,v [H=12, N=388, D=64]
  template queries (tokens 0:128) attend to template keys (0:128)
  search queries (tokens 128:388) attend to all 388 keys
  out = concat @ proj_w.T + proj_b

Kernel strategy per core (B_local=8 batches, all compute on device, bf16
matmuls with fp32 PSUM accumulation):
  - x cast to bf16, transposed feature-major via DMA-xbar transposes.
  - q^T,k^T = W^T-stationary matmuls (feature-major out, per-partition bias
    added in fp32 on ACT during the PSUM->SBUF copy).
  - v = x^T-stationary matmuls (token-major out), stored per-head with a ones
    column appended so the attention-value matmul also produces softmax sums.
  - scores computed TRANSPOSED: S^T[k,q] = k^T-slices as lhsT, q^T as rhs.
    exp on ACT (scale=1/8 folded in), probs in bf16.
  - AV: out^T[d,q] accumulated over k-chunks; row 64 = softmax denominators.
    AV for head h is emitted during head h+1's scores so the exps are ready.
  - normalize (deferred one more head): reciprocal on DVE (bf16), broadcast
    across 64 partitions via a [1,64]-ones PE matmul (gpsimd is slow per-op
    on HW), ACT-staged to SBUF, multiply on DVE (bf16 feature-major out).
  - proj matmul reads attention output directly (no transposes), bias on DVE,
    DMA out token-major fp32.
  - weight prep: fp32 DMA, DVE cast to bf16 (prefetched 2 chunks ahead),
    1-cycle/row PE transposes.
"""

import numpy as np

B, N, C = 64, 388, 768
H, D = 12, 64
LT = 128          # template tokens (= first token chunk, exactly)
LS = N - LT       # 260 search tokens
NCORES = 8
BL = B // NCORES  # 8 batches per core
O3 = 3 * C        # 2304
SCALE = 0.125
NPAD = 416        # 388 tokens padded to 32-multiple for DMA-transpose tiles

_NC_CACHE = {}


def _build_nc(dump=False, reps=1, skip=()):
    from contextlib import ExitStack

    import concourse.tile as tile
    from concourse import bacc, mybir
    from concourse.masks import make_identity

    f32 = mybir.dt.float32
    bf16 = mybir.dt.bfloat16
    Identity = mybir.ActivationFunctionType.Identity
    Exp = mybir.ActivationFunctionType.Exp
    mult = mybir.AluOpType.mult
    add = mybir.AluOpType.add

    nc = bacc.Bacc("TRN2", target_bir_lowering=False)

    x_ext = nc.dram_tensor("x", [BL, N, C], f32, kind="ExternalInput")
    qkvw_ext = nc.dram_tensor("qkv_w", [O3, C], f32, kind="ExternalInput")
    qkvb_ext = nc.dram_tensor("qkv_b", [O3], f32, kind="ExternalInput")
    projw_ext = nc.dram_tensor("proj_w", [C, C], f32, kind="ExternalInput")
    projb_ext = nc.dram_tensor("proj_b", [C], f32, kind="ExternalInput")
    out_ext = nc.dram_tensor("out", [BL, N, C], f32, kind="ExternalOutput")
    if dump:
        dxt_ext = nc.dram_tensor("d_xt", [128, 6, N], bf16, kind="ExternalOutput")
        dq_ext = nc.dram_tensor("d_q", [128, 6, N], bf16, kind="ExternalOutput")
        dk_ext = nc.dram_tensor("d_k", [128, 6, N], bf16, kind="ExternalOutput")
        dv_ext = nc.dram_tensor("d_v", [128, 4, H, 65], bf16, kind="ExternalOutput")
        da_ext = nc.dram_tensor("d_at", [128, 6, N], bf16, kind="ExternalOutput")
        dpav_ext = nc.dram_tensor("d_pav", [65, N], f32, kind="ExternalOutput")
        drb_ext = nc.dram_tensor("d_rb", [64, N], f32, kind="ExternalOutput")

    # token chunking of the 388 tokens: 128,128,128,4
    TCH = [(0, 128), (128, 128), (256, 128), (384, 4)]

    with tile.TileContext(nc) as tc, ExitStack() as ctx:
        const = ctx.enter_context(tc.tile_pool(name="const", bufs=1))
        stage = ctx.enter_context(tc.tile_pool(name="stage", bufs=4))
        # 8 PSUM banks total: 5 general + 2 deferred-AV accumulators + 1
        # reciprocal-broadcast target
        psum = ctx.enter_context(tc.tile_pool(name="ps", bufs=5, space="PSUM"))
        pavpool = ctx.enter_context(tc.tile_pool(name="pav", bufs=2, space="PSUM"))
        pbcpool = ctx.enter_context(tc.tile_pool(name="pbc", bufs=1, space="PSUM"))

        ident = const.tile([128, 128], f32)
        make_identity(nc, ident)
        ident_b = const.tile([128, 128], bf16)
        make_identity(nc, ident_b)
        ones1 = const.tile([1, 64], bf16)
        nc.vector.memset(ones1[:], 1.0)

        # ---- weights/biases declared here; emission interleaved with batch 0
        wT = const.tile([128, 6, O3], bf16)
        projT = const.tile([128, 6, C], bf16)
        qkb_sb = const.tile([128, 12], f32)
        vb_bc = const.tile([128, C], f32)
        pb_bc = const.tile([128, C], f32)

        def weights_gen():
            qb_st = stage.tile([12, 128], f32, tag="bst")
            nc.sync.dma_start(out=qb_st[:], in_=qkvb_ext[0:1536].rearrange("(j p) -> j p", p=128))
            pbt = psum.tile([128, 12], f32, tag="ps")
            nc.tensor.transpose(pbt[:], qb_st[:], ident[0:12, 0:12])
            nc.scalar.copy(out=qkb_sb[:], in_=pbt[:])

            # weight chunks: DMA fp32, cast bf16 on DVE, then 1-cycle/row PE
            # transposes. DMA+cast run two chunks ahead so the PE never waits
            # on the DMA->cast latency chain.
            wstbs = {}

            def fetch(j):
                wstf = stage.tile([128, C], f32, tag="wstf")
                src = qkvw_ext[j * 128:(j + 1) * 128, :] if j < 18 else \
                    projw_ext[(j - 18) * 128:(j - 17) * 128, :]
                nc.sync.dma_start(out=wstf[:], in_=src)
                wstb = stage.tile([128, C], bf16, tag="wstb")
                nc.vector.tensor_copy(out=wstb[:], in_=wstf[:])
                wstbs[j] = wstb

            fetch(0)
            fetch(1)
            for j in range(24):
                if j == 4:
                    nc.sync.dma_start(out=vb_bc[:], in_=qkvb_ext[1536:2304].unsqueeze(0).to_broadcast([128, C]))
                    nc.sync.dma_start(out=pb_bc[:], in_=projb_ext[:].unsqueeze(0).to_broadcast([128, C]))
                if j + 2 < 24:
                    fetch(j + 2)
                wstb = wstbs.pop(j)
                wdst = wT if j < 18 else projT
                jo = j * 128 if j < 18 else (j - 18) * 128
                for ct0, ng in ((0, 4), (4, 2)):
                    pt = psum.tile([128, ng * 128], bf16, tag="ps")
                    for g in range(ng):
                        ct = ct0 + g
                        nc.tensor.transpose(pt[:, g * 128:(g + 1) * 128],
                                            wstb[:, ct * 128:(ct + 1) * 128],
                                            ident_b[:])
                    dst = wdst[:, ct0:ct0 + ng, jo:jo + 128]
                    src_ap = pt[:].rearrange("p (g t) -> p g t", g=ng)
                    if ct0 == 0:
                        nc.scalar.copy(out=dst, in_=src_ap)
                    else:
                        nc.vector.tensor_copy(out=dst, in_=src_ap)
                yield

        # ---- per-batch pools ----
        xpool = ctx.enter_context(tc.tile_pool(name="xp", bufs=2))
        xtpool = ctx.enter_context(tc.tile_pool(name="xtp", bufs=2))
        qkpool = ctx.enter_context(tc.tile_pool(name="qkp", bufs=2))
        vpool = ctx.enter_context(tc.tile_pool(name="vp", bufs=2))
        apool = ctx.enter_context(tc.tile_pool(name="ap", bufs=2))
        ppool = ctx.enter_context(tc.tile_pool(name="pp", bufs=4))
        spool = ctx.enter_context(tc.tile_pool(name="ssp", bufs=4))
        opool = ctx.enter_context(tc.tile_pool(name="op", bufs=3))

        def emit_xload(b):
            # one DMA per 128-token chunk: the first transpose only waits for
            # chunk 0 (~1/3 of the full-x DMA time)
            xf = xpool.tile([128, 4, C], f32, tag="xf")
            for ti in range(3):
                nc.sync.dma_start(out=xf[:, ti, :],
                                  in_=x_ext[b, ti * 128:(ti + 1) * 128, :])
            nc.sync.dma_start(out=xf[0:4, 3, :], in_=x_ext[b, 384:388, :])
            return xf

        def stage1(b, xf, st):
            """Generator: transposes (4 items), q/k groups (12), v halves (8).
            Yields between PE-work units so attention of the previous batch
            can interleave. Fills `st` with the batch's tiles."""
            xTb = xtpool.tile([128, 6, N], bf16, tag="xt")
            st["xT"] = xTb
            xc = xpool.tile([128, 4, C], bf16, tag="xc")
            for ti, (t0, tp) in enumerate(TCH):
                # cast this chunk to bf16 on ACT, then 1-cycle/row transposes
                nc.scalar.copy(out=xc[0:tp, ti, :], in_=xf[0:tp, ti, :])
                for ct0, ng in ((0, 4), (4, 2)):
                    pt = psum.tile([128, ng * tp], bf16, tag="ps")
                    for g in range(ng):
                        ct = ct0 + g
                        nc.tensor.transpose(pt[:, g * tp:(g + 1) * tp],
                                            xc[0:tp, ti, ct * 128:(ct + 1) * 128],
                                            ident_b[0:tp, 0:tp])
                    dst = xTb[:, ct0:ct0 + ng, t0:t0 + tp]
                    src_ap = pt[:].rearrange("p (g t) -> p g t", g=ng)
                    if ct0 == 0:
                        nc.vector.tensor_copy(out=dst, in_=src_ap)
                    else:
                        nc.scalar.copy(out=dst, in_=src_ap)
                yield

            qTb = qkpool.tile([128, 6, N], bf16, tag="q")
            kTb = qkpool.tile([128, 6, N], bf16, tag="k")
            st["q"], st["k"] = qTb, kTb
            if "qkv" in skip:
                nc.vector.memset(qTb[:, 0:1, 0:2], 0.0)
                nc.vector.memset(kTb[:, 0:1, 0:2], 0.0)
            for j in range(12 if "qkv" not in skip else 0):
                ps = psum.tile([128, N], f32, tag="ps")
                for ct in range(6):
                    nc.tensor.matmul(
                        ps[:],
                        lhsT=wT[:, ct, j * 128:(j + 1) * 128],
                        rhs=xTb[:, ct, :],
                        start=(ct == 0), stop=(ct == 5),
                    )
                dst = qTb[:, j, :] if j < 6 else kTb[:, j - 6, :]
                if j % 2 == 0:
                    nc.scalar.activation(out=dst, in_=ps[:], func=Identity,
                                         bias=qkb_sb[:, j:j + 1], scale=1.0)
                else:
                    nc.vector.tensor_scalar(out=dst, in0=ps[:],
                                            scalar1=qkb_sb[:, j:j + 1], scalar2=None,
                                            op0=add)
                yield

            # block-diagonal remainder tiles for the 4 leftover key tokens:
            # kTrem[:, cth, 0:4] = even head's k-remainder (d-rows 0:64),
            # kTrem[:, cth, 32:36] = odd head's (d-rows 64:128); other columns
            # zero so one matmul yields both heads' remainder scores.
            kTrem = qkpool.tile([128, 6, 36], bf16, tag="krem")
            st["krem"] = kTrem
            nc.vector.memset(kTrem[:], 0.0)
            nc.vector.tensor_copy(out=kTrem[0:64, :, 0:4], in_=kTb[0:64, :, 384:388])
            nc.vector.tensor_copy(out=kTrem[64:128, :, 32:36], in_=kTb[64:128, :, 384:388])

            vb = vpool.tile([128, 4, H, 65], bf16, tag="v")
            st["v"] = vb
            nc.vector.memset(vb[:, :, :, 64:65], 1.0)
            for ti, (t0, tp) in enumerate(TCH if "qkv" not in skip else []):
                for o0, on, hs, he in ((0, 512, 0, 8), (512, 256, 8, 12)):
                    pv = psum.tile([128, on], f32, tag="ps")
                    for ct in range(6):
                        nc.tensor.matmul(
                            pv[0:tp, 0:on],
                            lhsT=xTb[:, ct, t0:t0 + tp],
                            rhs=wT[:, ct, 1536 + o0:1536 + o0 + on],
                            start=(ct == 0), stop=(ct == 5),
                        )
                    nc.vector.tensor_tensor(
                        out=vb[0:tp, ti, hs:he, 0:64],
                        in0=pv[0:tp, :].rearrange("p (h d) -> p h d", h=he - hs),
                        in1=vb_bc[0:tp, o0:o0 + on].rearrange("p (h d) -> p h d", h=he - hs),
                        op=add,
                    )
                    yield

            # v-remainder regrouped to match kTrem's partition layout
            vrem = vpool.tile([36, 6, 65], bf16, tag="vrem")
            st["vrem"] = vrem
            if "qkv" not in skip:
                nc.vector.tensor_copy(out=vrem[0:4, :, :], in_=vb[0:4, 3, 0:12:2, :])
                nc.vector.tensor_copy(out=vrem[32:36, :, :], in_=vb[0:4, 3, 1:12:2, :])
            else:
                nc.vector.memset(vrem[:], 0.0)
                nc.vector.memset(vb[:, :, :, 0:64], 0.0)

        def emit_attention(b, st, filler):
            """Attention heads; pulls filler items between scores and AVs.

            Softmax normalization is deferred one head: reciprocal of the
            denominator row (DVE, bf16) runs under the NEXT head's scores
            matmuls, then a [1,64]-ones PE matmul broadcasts it across 64
            partitions into PSUM, and the DVE multiply normalizes. This
            keeps gpsimd (slow per-op on HW) entirely out of the kernel.
            """
            qTb, kTb, vb = st["q"], st["k"], st["v"]
            kTrem, vrem = st["krem"], st["vrem"]
            xattnT = apool.tile([128, 6, N], bf16, tag="xat")
            st["at"] = xattnT
            if "attn" in skip:
                for _ct in range(6):
                    nc.vector.tensor_copy(out=xattnT[:, _ct, :], in_=wT[:, 0, 0:N])
            nheads = H if "attn" not in skip else 0
            pulled = 0

            def flush_epi(pend):
                pav_p, rinv_p, cth_p, r0_p, dump_p = pend
                pbc = pbcpool.tile([64, N], f32, tag="pbc")
                nc.tensor.matmul(pbc[:], lhsT=ones1[:], rhs=rinv_p[:],
                                 start=True, stop=True)
                # DVE can't read two PSUM operands; stage the broadcast in
                # SBUF via ACT (also keeps DVE free for the multiply)
                rbf = spool.tile([64, N], f32, tag="rbf")
                nc.scalar.copy(out=rbf[:], in_=pbc[:])
                if dump_p:
                    nc.sync.dma_start(out=drb_ext[:], in_=rbf[:])
                nc.vector.tensor_tensor(out=xattnT[r0_p:r0_p + 64, cth_p, :],
                                        in0=pav_p[0:64, :], in1=rbf[:], op=mult)

            def emit_av(h, probs, prr):
                """AV matmuls for head h (scores/exps were emitted one head
                earlier, so the exp outputs are ready — no PE stall)."""
                cth, r0 = h // 2, (h % 2) * 64
                pav = pavpool.tile([65, N], f32, tag="pav")
                nc.tensor.matmul(pav[:, 0:N], lhsT=vb[:, 0, h, :],
                                 rhs=probs[0][:, 0:N], start=True, stop=False)
                for kc, (t0, tp) in list(enumerate(TCH))[1:3]:
                    nc.tensor.matmul(pav[:, LT:N], lhsT=vb[0:tp, kc, h, :],
                                     rhs=probs[kc][0:tp, :],
                                     start=False, stop=False)
                rr = (h % 2) * 32
                nc.tensor.matmul(pav[:, LT:N], lhsT=vrem[rr:rr + 4, cth, :],
                                 rhs=prr[rr:rr + 4, :], start=False, stop=True)

                if dump and b == 0 and h == 0:
                    pavf = spool.tile([65, N], f32, tag="pavf")
                    nc.vector.tensor_copy(out=pavf[:], in_=pav[:])
                    nc.sync.dma_start(out=dpav_ext[:], in_=pavf[:])
                if "epi" in skip:
                    nc.vector.tensor_copy(out=xattnT[r0:r0 + 64, cth, :],
                                          in_=pav[0:64, :])
                    return None
                rinv = spool.tile([1, N], bf16, tag="ri")
                with nc.allow_low_precision(reason="bf16 1/denom feeds a "
                                            "ones-broadcast matmul; ~0.2% "
                                            "rel err, budget is 2e-2"):
                    nc.vector.reciprocal(out=rinv[:], in_=pav[64:65, :])
                return (pav, rinv, cth, r0, dump and b == 0 and h == 0)

            pend = None   # epilogue pending for head h-2
            prev = None   # (h, probs, prr) awaiting AV for head h-1
            for h in range(nheads):
                cth, r0 = h // 2, (h % 2) * 64
                qh = qTb[r0:r0 + 64, cth, :]   # [64, 388] bf16
                kh = kTb[r0:r0 + 64, cth, :]

                # all scores matmuls first; chunk 0 covers ALL queries
                # (template cols 0:128 + search 128:388); chunk 3 (the 4
                # remainder keys) is computed for the HEAD PAIR at even h via
                # the block-diagonal kTrem in one matmul + one exp
                probs = []
                for kc, (t0, tp) in list(enumerate(TCH))[0:3]:
                    pss = psum.tile([128, N if kc == 0 else LS], f32, tag="ps")
                    rhs_q = qh[:] if kc == 0 else qh[:, LT:N]
                    nc.tensor.matmul(pss[0:tp, :], lhsT=kh[:, t0:t0 + tp],
                                     rhs=rhs_q, start=True, stop=True)
                    prs = ppool.tile([128, N if kc == 0 else LS], bf16,
                                     tag="pr0" if kc == 0 else "prs")
                    nc.scalar.activation(out=prs[0:tp, :], in_=pss[0:tp, :],
                                         func=(Identity if "expid" in skip else Exp),
                                         scale=SCALE)
                    probs.append(prs)
                if h % 2 == 0:
                    psr = psum.tile([36, LS], f32, tag="ps")
                    nc.tensor.matmul(psr[:], lhsT=kTrem[:, cth, :],
                                     rhs=qTb[:, cth, LT:N], start=True, stop=True)
                    prr = ppool.tile([36, LS], bf16, tag="prr")
                    nc.scalar.activation(out=prr[:], in_=psr[:],
                                         func=(Identity if "expid" in skip else Exp),
                                         scale=SCALE)
                    st["prr"] = prr
                else:
                    prr = st["prr"]

                # filler work for neighbouring batches rides in the exp window
                want = (h + 1) * 22 // nheads
                while pulled < want and next(filler, "END") != "END":
                    pulled += 1

                # normalize head h-2, then AV for head h-1: both consume
                # results whose producers ran during this head's scores
                if pend is not None:
                    flush_epi(pend)
                    pend = None
                if prev is not None:
                    pend = emit_av(*prev)
                prev = (h, probs, prr)

            # tail: AV of the last head, then the two pending normalizes
            if prev is not None:
                pend2 = emit_av(*prev)
            if pend is not None:
                flush_epi(pend)
            if dump and b == 0:
                nc.sync.dma_start(out=dxt_ext[:], in_=st["xT"][:, :, 0:N])
                nc.sync.dma_start(out=dq_ext[:], in_=qTb[:])
                nc.sync.dma_start(out=dk_ext[:], in_=kTb[:])
                nc.sync.dma_start(out=dv_ext[:], in_=vb[:])
            # drain any remaining filler, then normalize the last head
            while next(filler, "END") != "END":
                pass
            if "attn" not in skip and pend2 is not None:
                flush_epi(pend2)
            if dump and b == 0:
                nc.sync.dma_start(out=da_ext[:], in_=xattnT[:])

        def proj_gen(b, st):
            """Generator: 4 proj+store chunk items."""
            xattnT = st["at"]
            for ti, (t0, tp) in enumerate(TCH if "proj" not in skip else []):
                osb = opool.tile([128, C], f32, tag="ob")
                for o0, on in ((0, 512), (512, 256)):
                    pp = psum.tile([128, on], f32, tag="ps")
                    for ct in range(6):
                        nc.tensor.matmul(
                            pp[0:tp, 0:on],
                            lhsT=xattnT[:, ct, t0:t0 + tp],
                            rhs=projT[:, ct, o0:o0 + on],
                            start=(ct == 0), stop=(ct == 5),
                        )
                    nc.vector.tensor_tensor(out=osb[0:tp, o0:o0 + on], in0=pp[0:tp, :],
                                            in1=pb_bc[0:tp, o0:o0 + on], op=add)
                nc.sync.dma_start(out=out_ext[b, t0:t0 + tp, :], in_=osb[0:tp, :])
                yield

        # ---- software-pipelined batch loop ----
        from itertools import chain

        seq = [bb for _ in range(reps) for bb in range(BL)]
        states = [dict() for _ in seq]
        # prologue: interleave the 24 weight-prep chunks with batch 0's
        # stage1 items (transposes need no weights; qk group j needs weight
        # chunk j which is already emitted by then; v needs chunks 12-17)
        gw = weights_gen()
        xf0 = emit_xload(seq[0])
        g0 = stage1(seq[0], xf0, states[0])
        # batch-0 x transposes first: PE starts gated only by the first
        # x-chunk DMA. After those 4 items the weight generator must lead,
        # so that v's second output half (g0 item 18) finds weight chunk 17
        # (gw item 18) already in PE program order.
        for i in range(24):
            if i < 4:
                next(g0, None)
                next(gw, None)
            else:
                next(gw, None)
                next(g0, None)
        for _ in g0:
            pass
        prev_proj = iter(())
        for i, b in enumerate(seq):
            if i + 1 < len(seq):
                xf_n = emit_xload(seq[i + 1])
                nxt = stage1(seq[i + 1], xf_n, states[i + 1])
            else:
                nxt = iter(())
            emit_attention(b, states[i], chain(prev_proj, nxt))
            prev_proj = proj_gen(b, states[i])
        for _ in prev_proj:
            pass

    nc.compile()
    return nc


def _get_nc():
    if "nc" not in _NC_CACHE:
        _NC_CACHE["nc"] = _build_nc()
    return _NC_CACHE["nc"]


def kernel(x, qkv_w, qkv_b, proj_w, proj_b, t_h=8, t_w=8, s_h=16, s_w=16):
    from concourse.bass_utils import run_bass_kernel_spmd

    x = np.ascontiguousarray(np.asarray(x, dtype=np.float32))
    qkv_w = np.ascontiguousarray(np.asarray(qkv_w, dtype=np.float32))
    qkv_b = np.ascontiguousarray(np.asarray(qkv_b, dtype=np.float32))
    proj_w = np.ascontiguousarray(np.asarray(proj_w, dtype=np.float32))
    proj_b = np.ascontiguousarray(np.asarray(proj_b, dtype=np.float32))

    nc = _get_nc()
    in_maps = [
        {
            "x": x[i * BL:(i + 1) * BL],
            "qkv_w": qkv_w,
            "qkv_b": qkv_b,
            "proj_w": proj_w,
            "proj_b": proj_b,
        }
        for i in range(NCORES)
    ]
    res = run_bass_kernel_spmd(nc, in_maps, core_ids=list(range(NCORES)))
    out = np.concatenate([res.results[i]["out"] for i in range(NCORES)], axis=0)
    return out.astype(np.float32)



# revision 29
# speedup vs baseline: 2.2176x; 1.2152x over previous
"""Sparse-attention Trainium2 kernel, 8-way data-parallel over batch.

Reference computation (per batch):
  qkv = x @ qkv_w.T + qkv_b              -> split q,k,v [H=12, N=388, D=64]
  template queries (tokens 0:128) attend to template keys (0:128)
  search queries (tokens 128:388) attend to all 388 keys
  out = concat @ proj_w.T + proj_b

Kernel strategy per core (B_local=8 batches, all compute on device, bf16
matmuls with fp32 PSUM accumulation):
  - x cast to bf16, transposed feature-major via DMA-xbar transposes.
  - q^T,k^T = W^T-stationary matmuls (feature-major out, per-partition bias
    added in fp32 on ACT during the PSUM->SBUF copy).
  - v = x^T-stationary matmuls (token-major out), stored per-head with a ones
    column appended so the attention-value matmul also produces softmax sums.
  - scores computed TRANSPOSED: S^T[k,q] = k^T-slices as lhsT, q^T as rhs.
    exp on ACT (scale=1/8 folded in), probs in bf16.
  - AV: out^T[d,q] accumulated over k-chunks; row 64 = softmax denominators.
    AV for head h is emitted during head h+1's scores so the exps are ready.
  - normalize (deferred one more head): reciprocal on DVE (bf16), broadcast
    across 64 partitions via a [1,64]-ones PE matmul (gpsimd is slow per-op
    on HW), ACT-staged to SBUF, multiply on DVE (bf16 feature-major out).
  - proj matmul reads attention output directly (no transposes), bias on DVE,
    DMA out token-major fp32.
  - weight prep: fp32 DMA, DVE cast to bf16 (prefetched 2 chunks ahead),
    1-cycle/row PE transposes.
"""

import numpy as np

B, N, C = 64, 388, 768
H, D = 12, 64
LT = 128          # template tokens (= first token chunk, exactly)
LS = N - LT       # 260 search tokens
NCORES = 8
BL = B // NCORES  # 8 batches per core
O3 = 3 * C        # 2304
SCALE = 0.125
NPAD = 416        # 388 tokens padded to 32-multiple for DMA-transpose tiles

_NC_CACHE = {}


def _build_nc(dump=False, reps=1, skip=()):
    from contextlib import ExitStack

    import concourse.tile as tile
    from concourse import bacc, mybir
    from concourse.masks import make_identity

    f32 = mybir.dt.float32
    bf16 = mybir.dt.bfloat16
    Identity = mybir.ActivationFunctionType.Identity
    Exp = mybir.ActivationFunctionType.Exp
    mult = mybir.AluOpType.mult
    add = mybir.AluOpType.add

    nc = bacc.Bacc("TRN2", target_bir_lowering=False)

    x_ext = nc.dram_tensor("x", [BL, N, C], f32, kind="ExternalInput")
    qkvw_ext = nc.dram_tensor("qkv_w", [O3, C], f32, kind="ExternalInput")
    qkvb_ext = nc.dram_tensor("qkv_b", [O3], f32, kind="ExternalInput")
    projw_ext = nc.dram_tensor("proj_w", [C, C], f32, kind="ExternalInput")
    projb_ext = nc.dram_tensor("proj_b", [C], f32, kind="ExternalInput")
    out_ext = nc.dram_tensor("out", [BL, N, C], f32, kind="ExternalOutput")
    if dump:
        dxt_ext = nc.dram_tensor("d_xt", [128, 6, N], bf16, kind="ExternalOutput")
        dq_ext = nc.dram_tensor("d_q", [128, 6, N], bf16, kind="ExternalOutput")
        dk_ext = nc.dram_tensor("d_k", [128, 6, N], bf16, kind="ExternalOutput")
        dv_ext = nc.dram_tensor("d_v", [128, 4, H, 65], bf16, kind="ExternalOutput")
        da_ext = nc.dram_tensor("d_at", [128, 6, N], bf16, kind="ExternalOutput")
        dpav_ext = nc.dram_tensor("d_pav", [65, N], f32, kind="ExternalOutput")
        drb_ext = nc.dram_tensor("d_rb", [64, N], f32, kind="ExternalOutput")

    # token chunking of the 388 tokens: 128,128,128,4
    TCH = [(0, 128), (128, 128), (256, 128), (384, 4)]

    with tile.TileContext(nc) as tc, ExitStack() as ctx:
        const = ctx.enter_context(tc.tile_pool(name="const", bufs=1))
        stage = ctx.enter_context(tc.tile_pool(name="stage", bufs=4))
        # 8 PSUM banks total: 4 general + 3 deferred-AV accumulators (pairs
        # flush two heads late) + 1 reciprocal-broadcast target
        psum = ctx.enter_context(tc.tile_pool(name="ps", bufs=4, space="PSUM"))
        pavpool = ctx.enter_context(tc.tile_pool(name="pav", bufs=3, space="PSUM"))
        pbcpool = ctx.enter_context(tc.tile_pool(name="pbc", bufs=1, space="PSUM"))

        ident = const.tile([128, 128], f32)
        make_identity(nc, ident)
        ident_b = const.tile([128, 128], bf16)
        make_identity(nc, ident_b)
        # block-diagonal ones for the pair-packed reciprocal broadcast:
        # row 0 -> out partitions 0:64 (even head), row 1 -> 64:128 (odd)
        ones2 = const.tile([2, 128], bf16)
        nc.vector.memset(ones2[:], 0.0)
        nc.vector.memset(ones2[0:1, 0:64], 1.0)
        nc.vector.memset(ones2[1:2, 64:128], 1.0)

        # ---- weights/biases declared here; emission interleaved with batch 0
        wT = const.tile([128, 6, O3], bf16)
        projT = const.tile([128, 6, C], bf16)
        qkb_sb = const.tile([128, 12], f32)
        vb_bc = const.tile([128, C], f32)
        pb_bc = const.tile([128, C], f32)

        def weights_gen():
            qb_st = stage.tile([12, 128], f32, tag="bst")
            nc.sync.dma_start(out=qb_st[:], in_=qkvb_ext[0:1536].rearrange("(j p) -> j p", p=128))
            pbt = psum.tile([128, 12], f32, tag="ps")
            nc.tensor.transpose(pbt[:], qb_st[:], ident[0:12, 0:12])
            nc.scalar.copy(out=qkb_sb[:], in_=pbt[:])

            # weight chunks: DMA fp32, cast bf16 on DVE, then 1-cycle/row PE
            # transposes. DMA+cast run two chunks ahead so the PE never waits
            # on the DMA->cast latency chain.
            wstbs = {}

            def fetch(j):
                wstf = stage.tile([128, C], f32, tag="wstf")
                src = qkvw_ext[j * 128:(j + 1) * 128, :] if j < 18 else \
                    projw_ext[(j - 18) * 128:(j - 17) * 128, :]
                nc.sync.dma_start(out=wstf[:], in_=src)
                wstb = stage.tile([128, C], bf16, tag="wstb")
                nc.vector.tensor_copy(out=wstb[:], in_=wstf[:])
                wstbs[j] = wstb

            fetch(0)
            fetch(1)
            for j in range(24):
                if j == 4:
                    nc.sync.dma_start(out=vb_bc[:], in_=qkvb_ext[1536:2304].unsqueeze(0).to_broadcast([128, C]))
                    nc.sync.dma_start(out=pb_bc[:], in_=projb_ext[:].unsqueeze(0).to_broadcast([128, C]))
                if j + 2 < 24:
                    fetch(j + 2)
                wstb = wstbs.pop(j)
                wdst = wT if j < 18 else projT
                jo = j * 128 if j < 18 else (j - 18) * 128
                for ct0, ng in ((0, 4), (4, 2)):
                    pt = psum.tile([128, ng * 128], bf16, tag="ps")
                    for g in range(ng):
                        ct = ct0 + g
                        nc.tensor.transpose(pt[:, g * 128:(g + 1) * 128],
                                            wstb[:, ct * 128:(ct + 1) * 128],
                                            ident_b[:])
                    dst = wdst[:, ct0:ct0 + ng, jo:jo + 128]
                    src_ap = pt[:].rearrange("p (g t) -> p g t", g=ng)
                    if ct0 == 0:
                        nc.scalar.copy(out=dst, in_=src_ap)
                    else:
                        nc.vector.tensor_copy(out=dst, in_=src_ap)
                yield

        # ---- per-batch pools ----
        xpool = ctx.enter_context(tc.tile_pool(name="xp", bufs=2))
        xtpool = ctx.enter_context(tc.tile_pool(name="xtp", bufs=2))
        qkpool = ctx.enter_context(tc.tile_pool(name="qkp", bufs=2))
        vpool = ctx.enter_context(tc.tile_pool(name="vp", bufs=2))
        apool = ctx.enter_context(tc.tile_pool(name="ap", bufs=2))
        ppool = ctx.enter_context(tc.tile_pool(name="pp", bufs=4))
        spool = ctx.enter_context(tc.tile_pool(name="ssp", bufs=4))
        opool = ctx.enter_context(tc.tile_pool(name="op", bufs=3))

        def emit_xload(b):
            # one DMA per 128-token chunk: the first transpose only waits for
            # chunk 0 (~1/3 of the full-x DMA time)
            xf = xpool.tile([128, 4, C], f32, tag="xf")
            for ti in range(3):
                nc.sync.dma_start(out=xf[:, ti, :],
                                  in_=x_ext[b, ti * 128:(ti + 1) * 128, :])
            nc.sync.dma_start(out=xf[0:4, 3, :], in_=x_ext[b, 384:388, :])
            return xf

        def stage1(b, xf, st):
            """Generator: transposes (4 items), q/k groups (12), v halves (8).
            Yields between PE-work units so attention of the previous batch
            can interleave. Fills `st` with the batch's tiles."""
            xTb = xtpool.tile([128, 6, N], bf16, tag="xt")
            st["xT"] = xTb
            xc = xpool.tile([128, 4, C], bf16, tag="xc")
            for ti, (t0, tp) in enumerate(TCH):
                # cast this chunk to bf16 on ACT, then 1-cycle/row transposes
                nc.scalar.copy(out=xc[0:tp, ti, :], in_=xf[0:tp, ti, :])
                for ct0, ng in ((0, 4), (4, 2)):
                    pt = psum.tile([128, ng * tp], bf16, tag="ps")
                    for g in range(ng):
                        ct = ct0 + g
                        nc.tensor.transpose(pt[:, g * tp:(g + 1) * tp],
                                            xc[0:tp, ti, ct * 128:(ct + 1) * 128],
                                            ident_b[0:tp, 0:tp])
                    dst = xTb[:, ct0:ct0 + ng, t0:t0 + tp]
                    src_ap = pt[:].rearrange("p (g t) -> p g t", g=ng)
                    if ct0 == 0:
                        nc.vector.tensor_copy(out=dst, in_=src_ap)
                    else:
                        nc.scalar.copy(out=dst, in_=src_ap)
                yield

            qTb = qkpool.tile([128, 6, N], bf16, tag="q")
            kTb = qkpool.tile([128, 6, N], bf16, tag="k")
            st["q"], st["k"] = qTb, kTb
            if "qkv" in skip:
                nc.vector.memset(qTb[:, 0:1, 0:2], 0.0)
                nc.vector.memset(kTb[:, 0:1, 0:2], 0.0)
            for j in range(12 if "qkv" not in skip else 0):
                ps = psum.tile([128, N], f32, tag="ps")
                for ct in range(6):
                    nc.tensor.matmul(
                        ps[:],
                        lhsT=wT[:, ct, j * 128:(j + 1) * 128],
                        rhs=xTb[:, ct, :],
                        start=(ct == 0), stop=(ct == 5),
                    )
                dst = qTb[:, j, :] if j < 6 else kTb[:, j - 6, :]
                if j % 2 == 0:
                    nc.scalar.activation(out=dst, in_=ps[:], func=Identity,
                                         bias=qkb_sb[:, j:j + 1], scale=1.0)
                else:
                    nc.vector.tensor_scalar(out=dst, in0=ps[:],
                                            scalar1=qkb_sb[:, j:j + 1], scalar2=None,
                                            op0=add)
                yield

            # block-diagonal remainder tiles for the 4 leftover key tokens:
            # kTrem[:, cth, 0:4] = even head's k-remainder (d-rows 0:64),
            # kTrem[:, cth, 32:36] = odd head's (d-rows 64:128); other columns
            # zero so one matmul yields both heads' remainder scores.
            kTrem = qkpool.tile([128, 6, 36], bf16, tag="krem")
            st["krem"] = kTrem
            nc.vector.memset(kTrem[:], 0.0)
            nc.vector.tensor_copy(out=kTrem[0:64, :, 0:4], in_=kTb[0:64, :, 384:388])
            nc.vector.tensor_copy(out=kTrem[64:128, :, 32:36], in_=kTb[64:128, :, 384:388])

            vb = vpool.tile([128, 4, H, 65], bf16, tag="v")
            st["v"] = vb
            nc.vector.memset(vb[:, :, :, 64:65], 1.0)
            for ti, (t0, tp) in enumerate(TCH if "qkv" not in skip else []):
                for o0, on, hs, he in ((0, 512, 0, 8), (512, 256, 8, 12)):
                    pv = psum.tile([128, on], f32, tag="ps")
                    for ct in range(6):
                        nc.tensor.matmul(
                            pv[0:tp, 0:on],
                            lhsT=xTb[:, ct, t0:t0 + tp],
                            rhs=wT[:, ct, 1536 + o0:1536 + o0 + on],
                            start=(ct == 0), stop=(ct == 5),
                        )
                    nc.vector.tensor_tensor(
                        out=vb[0:tp, ti, hs:he, 0:64],
                        in0=pv[0:tp, :].rearrange("p (h d) -> p h d", h=he - hs),
                        in1=vb_bc[0:tp, o0:o0 + on].rearrange("p (h d) -> p h d", h=he - hs),
                        op=add,
                    )
                    yield

            # v-remainder regrouped to match kTrem's partition layout
            vrem = vpool.tile([36, 6, 65], bf16, tag="vrem")
            st["vrem"] = vrem
            if "qkv" not in skip:
                nc.vector.tensor_copy(out=vrem[0:4, :, :], in_=vb[0:4, 3, 0:12:2, :])
                nc.vector.tensor_copy(out=vrem[32:36, :, :], in_=vb[0:4, 3, 1:12:2, :])
            else:
                nc.vector.memset(vrem[:], 0.0)
                nc.vector.memset(vb[:, :, :, 0:64], 0.0)

        def emit_attention(b, st, filler):
            """Attention heads; pulls filler items between scores and AVs.

            Softmax normalization is deferred one head: reciprocal of the
            denominator row (DVE, bf16) runs under the NEXT head's scores
            matmuls, then a [1,64]-ones PE matmul broadcasts it across 64
            partitions into PSUM, and the DVE multiply normalizes. This
            keeps gpsimd (slow per-op on HW) entirely out of the kernel.
            """
            qTb, kTb, vb = st["q"], st["k"], st["v"]
            kTrem, vrem = st["krem"], st["vrem"]
            xattnT = apool.tile([128, 6, N], bf16, tag="xat")
            st["at"] = xattnT
            if "attn" in skip:
                for _ct in range(6):
                    nc.vector.tensor_copy(out=xattnT[:, _ct, :], in_=wT[:, 0, 0:N])
            nheads = H if "attn" not in skip else 0
            pulled = 0

            def flush_epi(pend):
                """Normalize a HEAD PAIR: one [2,128] block-diag ones matmul
                broadcasts both reciprocal rows (even -> partitions 0:64,
                odd -> 64:128), one ACT copy stages it in SBUF (DVE can't
                read two PSUM operands), two DVE multiplies — the odd one
                reads in1 partition-shifted (64:128 vs out 0:64)."""
                pav_e, pav_o, rinv2_p, cth_p, dump_p = pend
                pbc = pbcpool.tile([128, N], f32, tag="pbc")
                nc.tensor.matmul(pbc[:], lhsT=ones2[:], rhs=rinv2_p[:],
                                 start=True, stop=True)
                rbf = spool.tile([128, N], f32, tag="rbf")
                nc.scalar.copy(out=rbf[:], in_=pbc[:])
                if dump_p:
                    nc.sync.dma_start(out=drb_ext[:], in_=rbf[0:64, :])
                nc.vector.tensor_tensor(out=xattnT[0:64, cth_p, :],
                                        in0=pav_e[0:64, :], in1=rbf[0:64, :],
                                        op=mult)
                nc.vector.tensor_tensor(out=xattnT[64:128, cth_p, :],
                                        in0=pav_o[0:64, :], in1=rbf[64:128, :],
                                        op=mult)

            def emit_av(h, probs, prr):
                """AV matmuls for head h (scores/exps were emitted one head
                earlier, so the exp outputs are ready — no PE stall)."""
                cth, r0 = h // 2, (h % 2) * 64
                pav = pavpool.tile([65, N], f32, tag="pav")
                nc.tensor.matmul(pav[:, 0:N], lhsT=vb[:, 0, h, :],
                                 rhs=probs[0][:, 0:N], start=True, stop=False)
                for kc, (t0, tp) in list(enumerate(TCH))[1:3]:
                    nc.tensor.matmul(pav[:, LT:N], lhsT=vb[0:tp, kc, h, :],
                                     rhs=probs[kc][0:tp, :],
                                     start=False, stop=False)
                rr = (h % 2) * 32
                nc.tensor.matmul(pav[:, LT:N], lhsT=vrem[rr:rr + 4, cth, :],
                                 rhs=prr[rr:rr + 4, :], start=False, stop=True)

                if dump and b == 0 and h == 0:
                    pavf = spool.tile([65, N], f32, tag="pavf")
                    nc.vector.tensor_copy(out=pavf[:], in_=pav[:])
                    nc.sync.dma_start(out=dpav_ext[:], in_=pavf[:])
                if "epi" in skip:
                    nc.vector.tensor_copy(out=xattnT[r0:r0 + 64, cth, :],
                                          in_=pav[0:64, :])
                    return None
                if h % 2 == 0:
                    rinv2 = spool.tile([2, N], bf16, tag="ri")
                    st["rinv2"] = rinv2
                else:
                    rinv2 = st["rinv2"]
                with nc.allow_low_precision(reason="bf16 1/denom feeds a "
                                            "ones-broadcast matmul; ~0.2% "
                                            "rel err, budget is 2e-2"):
                    nc.vector.reciprocal(out=rinv2[h % 2:h % 2 + 1, :],
                                         in_=pav[64:65, :])
                return pav

            pend = None       # pair epilogue pending (flushed at odd iters)
            pav_even = None   # even head's AV awaiting its pair partner
            prev = None       # (h, probs, prr) awaiting AV for head h-1
            for h in range(nheads):
                cth, r0 = h // 2, (h % 2) * 64
                qh = qTb[r0:r0 + 64, cth, :]   # [64, 388] bf16
                kh = kTb[r0:r0 + 64, cth, :]

                # all scores matmuls first; chunk 0 covers ALL queries
                # (template cols 0:128 + search 128:388); chunk 3 (the 4
                # remainder keys) is computed for the HEAD PAIR at even h via
                # the block-diagonal kTrem in one matmul + one exp
                probs = []
                for kc, (t0, tp) in list(enumerate(TCH))[0:3]:
                    pss = psum.tile([128, N if kc == 0 else LS], f32, tag="ps")
                    rhs_q = qh[:] if kc == 0 else qh[:, LT:N]
                    nc.tensor.matmul(pss[0:tp, :], lhsT=kh[:, t0:t0 + tp],
                                     rhs=rhs_q, start=True, stop=True)
                    prs = ppool.tile([128, N if kc == 0 else LS], bf16,
                                     tag="pr0" if kc == 0 else "prs")
                    nc.scalar.activation(out=prs[0:tp, :], in_=pss[0:tp, :],
                                         func=(Identity if "expid" in skip else Exp),
                                         scale=SCALE)
                    probs.append(prs)
                if h % 2 == 0:
                    psr = psum.tile([36, LS], f32, tag="ps")
                    nc.tensor.matmul(psr[:], lhsT=kTrem[:, cth, :],
                                     rhs=qTb[:, cth, LT:N], start=True, stop=True)
                    prr = ppool.tile([36, LS], bf16, tag="prr")
                    nc.scalar.activation(out=prr[:], in_=psr[:],
                                         func=(Identity if "expid" in skip else Exp),
                                         scale=SCALE)
                    st["prr"] = prr
                else:
                    prr = st["prr"]

                # filler work for neighbouring batches rides in the exp window
                want = (h + 1) * 22 // nheads
                while pulled < want and next(filler, "END") != "END":
                    pulled += 1

                # normalize the oldest complete pair (its reciprocals ran
                # during the previous head), then AV for head h-1
                if pend is not None and h % 2 == 1:
                    flush_epi(pend)
                    pend = None
                if prev is not None:
                    hp = prev[0]
                    pav = emit_av(*prev)
                    if hp % 2 == 0:
                        pav_even = pav
                    elif pav is not None:
                        pend = (pav_even, pav, st["rinv2"], hp // 2,
                                dump and b == 0 and hp == 1)
                prev = (h, probs, prr)

            # tail: AV of the last head completes the final pair
            pend2 = None
            if prev is not None:
                hp = prev[0]
                pav = emit_av(*prev)
                if pav is not None:
                    pend2 = (pav_even, pav, st["rinv2"], hp // 2, False)
            if pend is not None:
                flush_epi(pend)
            if dump and b == 0:
                nc.sync.dma_start(out=dxt_ext[:], in_=st["xT"][:, :, 0:N])
                nc.sync.dma_start(out=dq_ext[:], in_=qTb[:])
                nc.sync.dma_start(out=dk_ext[:], in_=kTb[:])
                nc.sync.dma_start(out=dv_ext[:], in_=vb[:])
            # drain any remaining filler, then normalize the last head
            while next(filler, "END") != "END":
                pass
            if "attn" not in skip and pend2 is not None:
                flush_epi(pend2)
            if dump and b == 0:
                nc.sync.dma_start(out=da_ext[:], in_=xattnT[:])

        def proj_gen(b, st):
            """Generator: 4 proj+store chunk items."""
            xattnT = st["at"]
            for ti, (t0, tp) in enumerate(TCH if "proj" not in skip else []):
                osb = opool.tile([128, C], f32, tag="ob")
                for o0, on in ((0, 512), (512, 256)):
                    pp = psum.tile([128, on], f32, tag="ps")
                    for ct in range(6):
                        nc.tensor.matmul(
                            pp[0:tp, 0:on],
                            lhsT=xattnT[:, ct, t0:t0 + tp],
                            rhs=projT[:, ct, o0:o0 + on],
                            start=(ct == 0), stop=(ct == 5),
                        )
                    nc.vector.tensor_tensor(out=osb[0:tp, o0:o0 + on], in0=pp[0:tp, :],
                                            in1=pb_bc[0:tp, o0:o0 + on], op=add)
                nc.sync.dma_start(out=out_ext[b, t0:t0 + tp, :], in_=osb[0:tp, :])
                yield

        # ---- software-pipelined batch loop ----
        from itertools import chain

        seq = [bb for _ in range(reps) for bb in range(BL)]
        states = [dict() for _ in seq]
        # prologue: interleave the 24 weight-prep chunks with batch 0's
        # stage1 items (transposes need no weights; qk group j needs weight
        # chunk j which is already emitted by then; v needs chunks 12-17)
        gw = weights_gen()
        xf0 = emit_xload(seq[0])
        g0 = stage1(seq[0], xf0, states[0])
        # batch-0 x transposes first: PE starts gated only by the first
        # x-chunk DMA. After those 4 items the weight generator must lead,
        # so that v's second output half (g0 item 18) finds weight chunk 17
        # (gw item 18) already in PE program order.
        for i in range(24):
            if i < 4:
                next(g0, None)
                next(gw, None)
            else:
                next(gw, None)
                next(g0, None)
        for _ in g0:
            pass
        prev_proj = iter(())
        for i, b in enumerate(seq):
            if i + 1 < len(seq):
                xf_n = emit_xload(seq[i + 1])
                nxt = stage1(seq[i + 1], xf_n, states[i + 1])
            else:
                nxt = iter(())
            emit_attention(b, states[i], chain(prev_proj, nxt))
            prev_proj = proj_gen(b, states[i])
        for _ in prev_proj:
            pass

    nc.compile()
    return nc


def _get_nc():
    if "nc" not in _NC_CACHE:
        _NC_CACHE["nc"] = _build_nc()
    return _NC_CACHE["nc"]


def kernel(x, qkv_w, qkv_b, proj_w, proj_b, t_h=8, t_w=8, s_h=16, s_w=16):
    from concourse.bass_utils import run_bass_kernel_spmd

    x = np.ascontiguousarray(np.asarray(x, dtype=np.float32))
    qkv_w = np.ascontiguousarray(np.asarray(qkv_w, dtype=np.float32))
    qkv_b = np.ascontiguousarray(np.asarray(qkv_b, dtype=np.float32))
    proj_w = np.ascontiguousarray(np.asarray(proj_w, dtype=np.float32))
    proj_b = np.ascontiguousarray(np.asarray(proj_b, dtype=np.float32))

    nc = _get_nc()
    in_maps = [
        {
            "x": x[i * BL:(i + 1) * BL],
            "qkv_w": qkv_w,
            "qkv_b": qkv_b,
            "proj_w": proj_w,
            "proj_b": proj_b,
        }
        for i in range(NCORES)
    ]
    res = run_bass_kernel_spmd(nc, in_maps, core_ids=list(range(NCORES)))
    out = np.concatenate([res.results[i]["out"] for i in range(NCORES)], axis=0)
    return out.astype(np.float32)



# revision 30
# speedup vs baseline: 2.2608x; 1.0195x over previous
"""Sparse-attention Trainium2 kernel, 8-way data-parallel over batch.

Reference computation (per batch):
  qkv = x @ qkv_w.T + qkv_b              -> split q,k,v [H=12, N=388, D=64]
  template queries (tokens 0:128) attend to template keys (0:128)
  search queries (tokens 128:388) attend to all 388 keys
  out = concat @ proj_w.T + proj_b

Kernel strategy per core (B_local=8 batches, all compute on device, bf16
matmuls with fp32 PSUM accumulation):
  - x cast to bf16, transposed feature-major via DMA-xbar transposes.
  - q^T,k^T = W^T-stationary matmuls (feature-major out, per-partition bias
    added in fp32 on ACT during the PSUM->SBUF copy).
  - v = x^T-stationary matmuls (token-major out), stored per-head with a ones
    column appended so the attention-value matmul also produces softmax sums.
  - scores computed TRANSPOSED: S^T[k,q] = k^T-slices as lhsT, q^T as rhs.
    exp on ACT (scale=1/8 folded in), probs in bf16.
  - AV: out^T[d,q] accumulated over k-chunks; row 64 = softmax denominators.
    AV for head h is emitted during head h+1's scores so the exps are ready.
  - normalize (deferred one more head): reciprocal on DVE (bf16), broadcast
    across 64 partitions via a [1,64]-ones PE matmul (gpsimd is slow per-op
    on HW), ACT-staged to SBUF, multiply on DVE (bf16 feature-major out).
  - proj matmul reads attention output directly (no transposes), bias on DVE,
    DMA out token-major fp32.
  - weight prep: fp32 DMA, DVE cast to bf16 (prefetched 2 chunks ahead),
    1-cycle/row PE transposes.
"""

import numpy as np

B, N, C = 64, 388, 768
H, D = 12, 64
LT = 128          # template tokens (= first token chunk, exactly)
LS = N - LT       # 260 search tokens
NCORES = 8
BL = B // NCORES  # 8 batches per core
O3 = 3 * C        # 2304
SCALE = 0.125
NPAD = 416        # 388 tokens padded to 32-multiple for DMA-transpose tiles

_NC_CACHE = {}


def _build_nc(dump=False, reps=1, skip=()):
    from contextlib import ExitStack

    import concourse.tile as tile
    from concourse import bacc, mybir
    from concourse.masks import make_identity

    f32 = mybir.dt.float32
    bf16 = mybir.dt.bfloat16
    Identity = mybir.ActivationFunctionType.Identity
    Exp = mybir.ActivationFunctionType.Exp
    mult = mybir.AluOpType.mult
    add = mybir.AluOpType.add

    nc = bacc.Bacc("TRN2", target_bir_lowering=False)

    x_ext = nc.dram_tensor("x", [BL, N, C], f32, kind="ExternalInput")
    qkvw_ext = nc.dram_tensor("qkv_w", [O3, C], f32, kind="ExternalInput")
    qkvb_ext = nc.dram_tensor("qkv_b", [O3], f32, kind="ExternalInput")
    projw_ext = nc.dram_tensor("proj_w", [C, C], f32, kind="ExternalInput")
    projb_ext = nc.dram_tensor("proj_b", [C], f32, kind="ExternalInput")
    out_ext = nc.dram_tensor("out", [BL, N, C], f32, kind="ExternalOutput")
    if dump:
        dxt_ext = nc.dram_tensor("d_xt", [128, 6, N], bf16, kind="ExternalOutput")
        dq_ext = nc.dram_tensor("d_q", [128, 6, N], bf16, kind="ExternalOutput")
        dk_ext = nc.dram_tensor("d_k", [128, 6, N], bf16, kind="ExternalOutput")
        dv_ext = nc.dram_tensor("d_v", [128, 4, H, 65], bf16, kind="ExternalOutput")
        da_ext = nc.dram_tensor("d_at", [128, 6, N], bf16, kind="ExternalOutput")
        dpav_ext = nc.dram_tensor("d_pav", [65, N], f32, kind="ExternalOutput")
        drb_ext = nc.dram_tensor("d_rb", [64, N], f32, kind="ExternalOutput")

    # token chunking of the 388 tokens: 128,128,128,4
    TCH = [(0, 128), (128, 128), (256, 128), (384, 4)]

    with tile.TileContext(nc) as tc, ExitStack() as ctx:
        const = ctx.enter_context(tc.tile_pool(name="const", bufs=1))
        stage = ctx.enter_context(tc.tile_pool(name="stage", bufs=4))
        # 8 PSUM banks total: 4 general + 3 deferred-AV accumulators (pairs
        # flush two heads late) + 1 reciprocal-broadcast target
        psum = ctx.enter_context(tc.tile_pool(name="ps", bufs=4, space="PSUM"))
        pavpool = ctx.enter_context(tc.tile_pool(name="pav", bufs=3, space="PSUM"))
        pbcpool = ctx.enter_context(tc.tile_pool(name="pbc", bufs=1, space="PSUM"))

        ident = const.tile([128, 128], f32)
        make_identity(nc, ident)
        ident_b = const.tile([128, 128], bf16)
        make_identity(nc, ident_b)
        # block-diagonal ones for the pair-packed reciprocal broadcast:
        # row 0 -> out partitions 0:64 (even head), row 1 -> 64:128 (odd)
        ones2 = const.tile([2, 128], bf16)
        nc.vector.memset(ones2[:], 0.0)
        nc.vector.memset(ones2[0:1, 0:64], 1.0)
        nc.vector.memset(ones2[1:2, 64:128], 1.0)

        # ---- weights/biases declared here; emission interleaved with batch 0
        wT = const.tile([128, 6, O3], bf16)
        projT = const.tile([128, 6, C], bf16)
        qkb_sb = const.tile([128, 12], f32)
        vb_bc = const.tile([128, C], f32)
        pb_bc = const.tile([128, C], f32)

        def weights_gen():
            qb_st = stage.tile([12, 128], f32, tag="bst")
            nc.sync.dma_start(out=qb_st[:], in_=qkvb_ext[0:1536].rearrange("(j p) -> j p", p=128))
            pbt = psum.tile([128, 12], f32, tag="ps")
            nc.tensor.transpose(pbt[:], qb_st[:], ident[0:12, 0:12])
            nc.scalar.copy(out=qkb_sb[:], in_=pbt[:])

            # weight chunks: DMA fp32, cast bf16 on DVE, then 1-cycle/row PE
            # transposes. DMA+cast run two chunks ahead so the PE never waits
            # on the DMA->cast latency chain.
            wstbs = {}

            def fetch(j):
                wstf = stage.tile([128, C], f32, tag="wstf")
                src = qkvw_ext[j * 128:(j + 1) * 128, :] if j < 18 else \
                    projw_ext[(j - 18) * 128:(j - 17) * 128, :]
                nc.sync.dma_start(out=wstf[:], in_=src)
                wstb = stage.tile([128, C], bf16, tag="wstb")
                nc.vector.tensor_copy(out=wstb[:], in_=wstf[:])
                wstbs[j] = wstb

            fetch(0)
            fetch(1)
            for j in range(24):
                if j == 4:
                    nc.sync.dma_start(out=vb_bc[:], in_=qkvb_ext[1536:2304].unsqueeze(0).to_broadcast([128, C]))
                    nc.sync.dma_start(out=pb_bc[:], in_=projb_ext[:].unsqueeze(0).to_broadcast([128, C]))
                if j + 2 < 24:
                    fetch(j + 2)
                wstb = wstbs.pop(j)
                wdst = wT if j < 18 else projT
                jo = j * 128 if j < 18 else (j - 18) * 128
                for ct0, ng in ((0, 4), (4, 2)):
                    pt = psum.tile([128, ng * 128], bf16, tag="ps")
                    for g in range(ng):
                        ct = ct0 + g
                        nc.tensor.transpose(pt[:, g * 128:(g + 1) * 128],
                                            wstb[:, ct * 128:(ct + 1) * 128],
                                            ident_b[:])
                    dst = wdst[:, ct0:ct0 + ng, jo:jo + 128]
                    src_ap = pt[:].rearrange("p (g t) -> p g t", g=ng)
                    if ct0 == 0:
                        nc.scalar.copy(out=dst, in_=src_ap)
                    else:
                        nc.vector.tensor_copy(out=dst, in_=src_ap)
                yield

        # ---- per-batch pools ----
        xpool = ctx.enter_context(tc.tile_pool(name="xp", bufs=2))
        xtpool = ctx.enter_context(tc.tile_pool(name="xtp", bufs=2))
        qkpool = ctx.enter_context(tc.tile_pool(name="qkp", bufs=2))
        vpool = ctx.enter_context(tc.tile_pool(name="vp", bufs=2))
        apool = ctx.enter_context(tc.tile_pool(name="ap", bufs=2))
        ppool = ctx.enter_context(tc.tile_pool(name="pp", bufs=4))
        spool = ctx.enter_context(tc.tile_pool(name="ssp", bufs=4))
        opool = ctx.enter_context(tc.tile_pool(name="op", bufs=3))

        def emit_xload(b):
            # one DMA per 128-token chunk: the first transpose only waits for
            # chunk 0 (~1/3 of the full-x DMA time)
            xf = xpool.tile([128, 4, C], f32, tag="xf")
            for ti in range(3):
                nc.sync.dma_start(out=xf[:, ti, :],
                                  in_=x_ext[b, ti * 128:(ti + 1) * 128, :])
            nc.sync.dma_start(out=xf[0:4, 3, :], in_=x_ext[b, 384:388, :])
            return xf

        def stage1(b, xf, st):
            """Generator: transposes (4 items), q/k groups (12), v halves (8).
            Yields between PE-work units so attention of the previous batch
            can interleave. Fills `st` with the batch's tiles."""
            xTb = xtpool.tile([128, 6, N], bf16, tag="xt")
            st["xT"] = xTb
            xc = xpool.tile([128, 4, C], bf16, tag="xc")
            for ti, (t0, tp) in enumerate(TCH):
                # cast this chunk to bf16 on ACT, then 1-cycle/row transposes
                nc.scalar.copy(out=xc[0:tp, ti, :], in_=xf[0:tp, ti, :])
                for ct0, ng in ((0, 4), (4, 2)):
                    pt = psum.tile([128, ng * tp], bf16, tag="ps")
                    for g in range(ng):
                        ct = ct0 + g
                        nc.tensor.transpose(pt[:, g * tp:(g + 1) * tp],
                                            xc[0:tp, ti, ct * 128:(ct + 1) * 128],
                                            ident_b[0:tp, 0:tp])
                    dst = xTb[:, ct0:ct0 + ng, t0:t0 + tp]
                    src_ap = pt[:].rearrange("p (g t) -> p g t", g=ng)
                    if ct0 == 0:
                        nc.vector.tensor_copy(out=dst, in_=src_ap)
                    else:
                        nc.scalar.copy(out=dst, in_=src_ap)
                yield

            qTb = qkpool.tile([128, 6, N], bf16, tag="q")
            kTb = qkpool.tile([128, 6, N], bf16, tag="k")
            st["q"], st["k"] = qTb, kTb
            if "qkv" in skip:
                nc.vector.memset(qTb[:, 0:1, 0:2], 0.0)
                nc.vector.memset(kTb[:, 0:1, 0:2], 0.0)
            for j in range(12 if "qkv" not in skip else 0):
                ps = psum.tile([128, N], f32, tag="ps")
                for ct in range(6):
                    nc.tensor.matmul(
                        ps[:],
                        lhsT=wT[:, ct, j * 128:(j + 1) * 128],
                        rhs=xTb[:, ct, :],
                        start=(ct == 0), stop=(ct == 5),
                    )
                dst = qTb[:, j, :] if j < 6 else kTb[:, j - 6, :]
                if j % 2 == 0:
                    nc.scalar.activation(out=dst, in_=ps[:], func=Identity,
                                         bias=qkb_sb[:, j:j + 1], scale=1.0)
                else:
                    nc.vector.tensor_scalar(out=dst, in0=ps[:],
                                            scalar1=qkb_sb[:, j:j + 1], scalar2=None,
                                            op0=add)
                yield

            # block-diagonal remainder tiles for the 4 leftover key tokens:
            # kTrem[:, cth, 0:4] = even head's k-remainder (d-rows 0:64),
            # kTrem[:, cth, 32:36] = odd head's (d-rows 64:128); other columns
            # zero so one matmul yields both heads' remainder scores.
            kTrem = qkpool.tile([128, 6, 36], bf16, tag="krem")
            st["krem"] = kTrem
            nc.vector.memset(kTrem[:], 0.0)
            nc.vector.tensor_copy(out=kTrem[0:64, :, 0:4], in_=kTb[0:64, :, 384:388])
            nc.vector.tensor_copy(out=kTrem[64:128, :, 32:36], in_=kTb[64:128, :, 384:388])

            vb = vpool.tile([128, 4, H, 65], bf16, tag="v")
            st["v"] = vb
            nc.vector.memset(vb[:, :, :, 64:65], 1.0)
            for ti, (t0, tp) in enumerate(TCH if "qkv" not in skip else []):
                for o0, on, hs, he in ((0, 512, 0, 8), (512, 256, 8, 12)):
                    pv = psum.tile([128, on], f32, tag="ps")
                    for ct in range(6):
                        nc.tensor.matmul(
                            pv[0:tp, 0:on],
                            lhsT=xTb[:, ct, t0:t0 + tp],
                            rhs=wT[:, ct, 1536 + o0:1536 + o0 + on],
                            start=(ct == 0), stop=(ct == 5),
                        )
                    nc.vector.tensor_tensor(
                        out=vb[0:tp, ti, hs:he, 0:64],
                        in0=pv[0:tp, :].rearrange("p (h d) -> p h d", h=he - hs),
                        in1=vb_bc[0:tp, o0:o0 + on].rearrange("p (h d) -> p h d", h=he - hs),
                        op=add,
                    )
                    yield

            # v-remainder regrouped to match kTrem's partition layout
            vrem = vpool.tile([36, 6, 65], bf16, tag="vrem")
            st["vrem"] = vrem
            if "qkv" not in skip:
                nc.vector.tensor_copy(out=vrem[0:4, :, :], in_=vb[0:4, 3, 0:12:2, :])
                nc.vector.tensor_copy(out=vrem[32:36, :, :], in_=vb[0:4, 3, 1:12:2, :])
            else:
                nc.vector.memset(vrem[:], 0.0)
                nc.vector.memset(vb[:, :, :, 0:64], 0.0)

        def emit_attention(b, st, filler):
            """Attention heads; pulls filler items between scores and AVs.

            Softmax normalization is deferred one head: reciprocal of the
            denominator row (DVE, bf16) runs under the NEXT head's scores
            matmuls, then a [1,64]-ones PE matmul broadcasts it across 64
            partitions into PSUM, and the DVE multiply normalizes. This
            keeps gpsimd (slow per-op on HW) entirely out of the kernel.
            """
            qTb, kTb, vb = st["q"], st["k"], st["v"]
            kTrem, vrem = st["krem"], st["vrem"]
            xattnT = apool.tile([128, 6, N], bf16, tag="xat")
            st["at"] = xattnT
            if "attn" in skip:
                for _ct in range(6):
                    nc.vector.tensor_copy(out=xattnT[:, _ct, :], in_=wT[:, 0, 0:N])
            nheads = H if "attn" not in skip else 0
            pulled = 0

            def flush_epi(pend):
                """Normalize a HEAD PAIR: one [2,128] block-diag ones matmul
                broadcasts both reciprocal rows (even -> partitions 0:64,
                odd -> 64:128), one ACT copy stages it in SBUF (DVE can't
                read two PSUM operands), two DVE multiplies — the odd one
                reads in1 partition-shifted (64:128 vs out 0:64)."""
                pav_e, pav_o, rinv2_p, cth_p, dump_p = pend
                pbc = pbcpool.tile([128, N], f32, tag="pbc")
                nc.tensor.matmul(pbc[:], lhsT=ones2[:], rhs=rinv2_p[:],
                                 start=True, stop=True)
                rbf = spool.tile([128, N], f32, tag="rbf")
                nc.scalar.copy(out=rbf[:], in_=pbc[:])
                if dump_p:
                    nc.sync.dma_start(out=drb_ext[:], in_=rbf[0:64, :])
                nc.vector.tensor_tensor(out=xattnT[0:64, cth_p, :],
                                        in0=pav_e[0:64, :], in1=rbf[0:64, :],
                                        op=mult)
                nc.vector.tensor_tensor(out=xattnT[64:128, cth_p, :],
                                        in0=pav_o[0:64, :], in1=rbf[64:128, :],
                                        op=mult)

            def emit_av(h, probs, prr):
                """AV matmuls for head h (scores/exps were emitted one head
                earlier, so the exp outputs are ready — no PE stall)."""
                cth, r0 = h // 2, (h % 2) * 64
                pav = pavpool.tile([65, N], f32, tag="pav")
                nc.tensor.matmul(pav[:, 0:N], lhsT=vb[:, 0, h, :],
                                 rhs=probs[0][:, 0:N], start=True, stop=False)
                for kc, (t0, tp) in list(enumerate(TCH))[1:3]:
                    nc.tensor.matmul(pav[:, LT:N], lhsT=vb[0:tp, kc, h, :],
                                     rhs=probs[kc][0:tp, :],
                                     start=False, stop=False)
                rr = (h % 2) * 32
                nc.tensor.matmul(pav[:, LT:N], lhsT=vrem[rr:rr + 4, cth, :],
                                 rhs=prr[rr:rr + 4, :], start=False, stop=True)

                if dump and b == 0 and h == 0:
                    pavf = spool.tile([65, N], f32, tag="pavf")
                    nc.vector.tensor_copy(out=pavf[:], in_=pav[:])
                    nc.sync.dma_start(out=dpav_ext[:], in_=pavf[:])
                if "epi" in skip:
                    nc.vector.tensor_copy(out=xattnT[r0:r0 + 64, cth, :],
                                          in_=pav[0:64, :])
                    return None
                if h % 2 == 0:
                    rinv2 = spool.tile([2, N], bf16, tag="ri")
                    st["rinv2"] = rinv2
                else:
                    rinv2 = st["rinv2"]
                with nc.allow_low_precision(reason="bf16 1/denom feeds a "
                                            "ones-broadcast matmul; ~0.2% "
                                            "rel err, budget is 2e-2"):
                    nc.vector.reciprocal(out=rinv2[h % 2:h % 2 + 1, :],
                                         in_=pav[64:65, :])
                return pav

            pend = None       # pair epilogue pending (flushed at odd iters)
            pav_even = None   # even head's AV awaiting its pair partner
            prev = None       # (h, probs, prr) awaiting AV for head h-1
            for h in range(nheads):
                cth, r0 = h // 2, (h % 2) * 64
                qh = qTb[r0:r0 + 64, cth, :]   # [64, 388] bf16
                kh = kTb[r0:r0 + 64, cth, :]

                # all scores matmuls first; chunk 0 covers ALL queries
                # (template cols 0:128 + search 128:388); chunk 3 (the 4
                # remainder keys) is computed for the HEAD PAIR at even h via
                # the block-diagonal kTrem in one matmul + one exp
                probs = []
                for kc, (t0, tp) in list(enumerate(TCH))[0:3]:
                    pss = psum.tile([128, N if kc == 0 else LS], f32, tag="ps")
                    rhs_q = qh[:] if kc == 0 else qh[:, LT:N]
                    nc.tensor.matmul(pss[0:tp, :], lhsT=kh[:, t0:t0 + tp],
                                     rhs=rhs_q, start=True, stop=True)
                    prs = ppool.tile([128, N if kc == 0 else LS], bf16,
                                     tag="pr0" if kc == 0 else "prs")
                    nc.scalar.activation(out=prs[0:tp, :], in_=pss[0:tp, :],
                                         func=(Identity if "expid" in skip else Exp),
                                         scale=SCALE)
                    probs.append(prs)
                if h % 2 == 0:
                    psr = psum.tile([36, LS], f32, tag="ps")
                    nc.tensor.matmul(psr[:], lhsT=kTrem[:, cth, :],
                                     rhs=qTb[:, cth, LT:N], start=True, stop=True)
                    prr = ppool.tile([36, LS], bf16, tag="prr")
                    nc.scalar.activation(out=prr[:], in_=psr[:],
                                         func=(Identity if "expid" in skip else Exp),
                                         scale=SCALE)
                    st["prr"] = prr
                else:
                    prr = st["prr"]

                # filler work for neighbouring batches rides in the exp window
                want = (h + 1) * 22 // nheads
                while pulled < want and next(filler, "END") != "END":
                    pulled += 1

                # normalize the oldest complete pair (its reciprocals ran
                # during the previous head), then AV for head h-1
                if pend is not None and h % 2 == 1:
                    flush_epi(pend)
                    pend = None
                if prev is not None:
                    hp = prev[0]
                    pav = emit_av(*prev)
                    if hp % 2 == 0:
                        pav_even = pav
                    elif pav is not None:
                        pend = (pav_even, pav, st["rinv2"], hp // 2,
                                dump and b == 0 and hp == 1)
                prev = (h, probs, prr)

            # tail: AV of the last head completes the final pair
            pend2 = None
            if prev is not None:
                hp = prev[0]
                pav = emit_av(*prev)
                if pav is not None:
                    pend2 = (pav_even, pav, st["rinv2"], hp // 2, False)
            if pend is not None:
                flush_epi(pend)
            if dump and b == 0:
                nc.sync.dma_start(out=dxt_ext[:], in_=st["xT"][:, :, 0:N])
                nc.sync.dma_start(out=dq_ext[:], in_=qTb[:])
                nc.sync.dma_start(out=dk_ext[:], in_=kTb[:])
                nc.sync.dma_start(out=dv_ext[:], in_=vb[:])
            # drain any remaining filler, then normalize the last head
            while next(filler, "END") != "END":
                pass
            if "attn" not in skip and pend2 is not None:
                flush_epi(pend2)
            if dump and b == 0:
                nc.sync.dma_start(out=da_ext[:], in_=xattnT[:])

        prem_state = {}

        def proj_gen(b, st):
            """Generator: 3 full proj chunk items, plus a PAIRED flush of the
            4-token remainders: the token-major matmul cost is the streamed
            W-columns (768x6) regardless of token count, so two batches'
            remainder tokens share one matmul pass (halves that cost)."""
            if "proj" in skip:
                return
            xattnT = st["at"]
            for ti, (t0, tp) in enumerate(TCH[0:3]):
                osb = opool.tile([128, C], f32, tag="ob")
                for o0, on in ((0, 512), (512, 256)):
                    pp = psum.tile([128, on], f32, tag="ps")
                    for ct in range(6):
                        nc.tensor.matmul(
                            pp[0:tp, 0:on],
                            lhsT=xattnT[:, ct, t0:t0 + tp],
                            rhs=projT[:, ct, o0:o0 + on],
                            start=(ct == 0), stop=(ct == 5),
                        )
                    nc.vector.tensor_tensor(out=osb[0:tp, o0:o0 + on], in0=pp[0:tp, :],
                                            in1=pb_bc[0:tp, o0:o0 + on], op=add)
                nc.sync.dma_start(out=out_ext[b, t0:t0 + tp, :], in_=osb[0:tp, :])
                yield
            # stage this batch's remainder columns; flush on every 2nd batch
            if "pr" not in prem_state:
                prem = spool.tile([128, 6, 8], bf16, tag="prem")
                nc.vector.tensor_copy(out=prem[:, :, 0:4], in_=xattnT[:, :, 384:388])
                prem_state["pr"] = prem
                prem_state["b0"] = b
                return
            prem = prem_state.pop("pr")
            b0 = prem_state.pop("b0")
            nc.vector.tensor_copy(out=prem[:, :, 4:8], in_=xattnT[:, :, 384:388])
            osb = opool.tile([8, C], f32, tag="obr")
            for o0, on in ((0, 512), (512, 256)):
                pp = psum.tile([8, on], f32, tag="ps")
                for ct in range(6):
                    nc.tensor.matmul(
                        pp[:, 0:on],
                        lhsT=prem[:, ct, :],
                        rhs=projT[:, ct, o0:o0 + on],
                        start=(ct == 0), stop=(ct == 5),
                    )
                nc.vector.tensor_tensor(out=osb[:, o0:o0 + on], in0=pp[:],
                                        in1=pb_bc[0:8, o0:o0 + on], op=add)
            nc.sync.dma_start(out=out_ext[b0, 384:388, :], in_=osb[0:4, :])
            nc.sync.dma_start(out=out_ext[b, 384:388, :], in_=osb[4:8, :])
            yield

        # ---- software-pipelined batch loop ----
        from itertools import chain

        seq = [bb for _ in range(reps) for bb in range(BL)]
        states = [dict() for _ in seq]
        # prologue: interleave the 24 weight-prep chunks with batch 0's
        # stage1 items (transposes need no weights; qk group j needs weight
        # chunk j which is already emitted by then; v needs chunks 12-17)
        gw = weights_gen()
        xf0 = emit_xload(seq[0])
        g0 = stage1(seq[0], xf0, states[0])
        # batch-0 x transposes first: PE starts gated only by the first
        # x-chunk DMA. After those 4 items the weight generator must lead,
        # so that v's second output half (g0 item 18) finds weight chunk 17
        # (gw item 18) already in PE program order.
        for i in range(24):
            if i < 4:
                next(g0, None)
                next(gw, None)
            else:
                next(gw, None)
                next(g0, None)
        for _ in g0:
            pass
        prev_proj = iter(())
        for i, b in enumerate(seq):
            if i + 1 < len(seq):
                xf_n = emit_xload(seq[i + 1])
                nxt = stage1(seq[i + 1], xf_n, states[i + 1])
            else:
                nxt = iter(())
            emit_attention(b, states[i], chain(prev_proj, nxt))
            prev_proj = proj_gen(b, states[i])
        for _ in prev_proj:
            pass

    nc.compile()
    return nc


def _get_nc():
    if "nc" not in _NC_CACHE:
        _NC_CACHE["nc"] = _build_nc()
    return _NC_CACHE["nc"]


def kernel(x, qkv_w, qkv_b, proj_w, proj_b, t_h=8, t_w=8, s_h=16, s_w=16):
    from concourse.bass_utils import run_bass_kernel_spmd

    x = np.ascontiguousarray(np.asarray(x, dtype=np.float32))
    qkv_w = np.ascontiguousarray(np.asarray(qkv_w, dtype=np.float32))
    qkv_b = np.ascontiguousarray(np.asarray(qkv_b, dtype=np.float32))
    proj_w = np.ascontiguousarray(np.asarray(proj_w, dtype=np.float32))
    proj_b = np.ascontiguousarray(np.asarray(proj_b, dtype=np.float32))

    nc = _get_nc()
    in_maps = [
        {
            "x": x[i * BL:(i + 1) * BL],
            "qkv_w": qkv_w,
            "qkv_b": qkv_b,
            "proj_w": proj_w,
            "proj_b": proj_b,
        }
        for i in range(NCORES)
    ]
    res = run_bass_kernel_spmd(nc, in_maps, core_ids=list(range(NCORES)))
    out = np.concatenate([res.results[i]["out"] for i in range(NCORES)], axis=0)
    return out.astype(np.float32)

